# revision 1
# baseline (speedup 1.0000x reference)
"""MultiHeadAttention TRN2 Bass kernel (8 NeuronCores).

Sharding: core c = (batch b = c//2, query-half = c%2). Each core computes
K/V for its full batch (2048 keys) and attention + output projection + LN
for its 1024 query rows. No collectives; host gathers per-core outputs.

Device math (all matmuls in float32r = full-rate fp32, rel err ~2e-4):
  QhT[hd, q]  = wq[d, hd].T @ qT[d, q]          (per 4-head phase)
  KhT[hd, m]  = wk[d, hd].T @ qT[d, m]
  Vaug[m, 65] = qT[d, m].T @ wv[d, 65-packed]   (+ ones column)
  S^T[m, q]   = KhT[dk, m].T @ QhT[dk, q]       (K=64, head pairs packed
                                                 via tile_position rows)
  E = exp(S^T / 32)                              (ACT, PSUM->SBUF fp32r)
  OT[65, q]   = sum_m Vaug[m,65].T @ E[m, q]    (row 64 = softmax denom)
  CT[c, q]    = OT[0:64] * (1/denom)            (K=1 ones-matmul bcast +
                                                 reciprocal_approx_fast)
  Y[q, o]     = CT[c, q].T @ pwT[c, o] + (q_res + proj_b)   then LayerNorm
"""
import numpy as np

import concourse.bass as bass
import concourse.mybir as mybir
import concourse.tile as tile
from concourse import bacc
from concourse.bass_utils import run_bass_kernel_spmd

F32 = mybir.dt.float32
F32R = mybir.dt.float32r
AF = mybir.ActivationFunctionType
ALU = mybir.AluOpType
AX = mybir.AxisListType

B, L, D = 4, 2048, 1024
H, DK = 16, 64
HALF = 1024            # query rows per core
TEMPER = 32.0          # sqrt(d_model)
PHASES = 4
HP = H // PHASES       # 4 heads per phase
PAIRS = HP // 2        # 2 head-pairs per phase
MT = L // 128          # 16 m-tiles
LN_EPS = 1e-3

_CACHE = {}


def build(iters=1):
    nc = bacc.Bacc(None, target_bir_lowering=False)
    qt_d = nc.dram_tensor("qt", [D, L], F32R, kind="ExternalInput")
    qres_d = nc.dram_tensor("qres", [HALF, D], F32, kind="ExternalInput")
    wq_d = nc.dram_tensor("wq", [D, H * DK], F32R, kind="ExternalInput")
    wk_d = nc.dram_tensor("wk", [D, H * DK], F32R, kind="ExternalInput")
    wv_d = nc.dram_tensor("wv", [D, H * 65], F32R, kind="ExternalInput")
    pw_d = nc.dram_tensor("pw", [D, D], F32R, kind="ExternalInput")
    lna_d = nc.dram_tensor("lna", [1, D], F32, kind="ExternalInput")
    lnb_d = nc.dram_tensor("lnb", [1, D], F32, kind="ExternalInput")
    ones_d = nc.dram_tensor("ones64", [1, 64], F32R, kind="ExternalInput")
    out_d = nc.dram_tensor("out", [HALF, D], F32, kind="ExternalOutput")

    with tile.TileContext(nc) as tc:
        with (
            tc.tile_pool(name="p1", bufs=1) as p1,
            tc.tile_pool(name="p2", bufs=2) as p2,
            tc.tile_pool(name="psA", bufs=4, space="PSUM") as psA,
            tc.tile_pool(name="psS", bufs=2, space="PSUM") as psS,
        ):
            # ---- one-time constants ----
            ones_t = p1.tile([128, 64], F32R, name="ones_t")
            nc.sync.dma_start(ones_t[64:65, :], ones_d[:])
            ones_sb = p1.tile([128, 16], F32R, name="ones_sb")
            nc.sync.dma_start(ones_sb[:], ones_d[:, 0:16].to_broadcast([128, 16]))
            lna_t = p1.tile([128, D], F32, name="lna_t")
            nc.sync.dma_start(lna_t[:], lna_d[:].to_broadcast([128, D]))
            lnb_t = p1.tile([128, D], F32, name="lnb_t")
            nc.sync.dma_start(lnb_t[:], lnb_d[:].to_broadcast([128, D]))

            ct_t = p1.tile([128, H // 2, HALF], F32R, name="ct_t")

            for it in range(iters):
                for p in range(PHASES):
                    c0 = p * HP * DK
                    wq_t = p1.tile([128, 8, HP * DK], F32R, name=f"it{it}_wq_{p}", tag="wq")
                    nc.sync.dma_start(
                        wq_t[:],
                        wq_d[:, c0:c0 + HP * DK].rearrange("(dj pp) f -> pp dj f", pp=128),
                    )
                    wk_t = p1.tile([128, 8, HP * DK], F32R, name=f"it{it}_wk_{p}", tag="wk")
                    nc.sync.dma_start(
                        wk_t[:],
                        wk_d[:, c0:c0 + HP * DK].rearrange("(dj pp) f -> pp dj f", pp=128),
                    )
                    v0 = p * HP * 65
                    wv_t = p1.tile([128, 8, HP * 65], F32R, name=f"it{it}_wv_{p}", tag="wv")
                    nc.sync.dma_start(
                        wv_t[:],
                        wv_d[:, v0:v0 + HP * 65].rearrange("(dj pp) f -> pp dj f", pp=128),
                    )

                    qht_t = p2.tile([128, PAIRS, HALF], F32R, name=f"it{it}_qht_{p}", tag="qht")
                    kht_t = p2.tile([128, PAIRS, L], F32R, name=f"it{it}_kht_{p}", tag="kht")
                    vaug_t = p2.tile([128, MT, HP * 65], F32R, name=f"it{it}_vaug_{p}", tag="vaug")

                    # ---- QKV projections, streaming qT in 512-col blocks ----
                    for mc in range(L // 512):
                        qt_t = p2.tile([128, 8, 512], F32R, name=f"it{it}_qt_{p}_{mc}", tag="qt")
                        nc.sync.dma_start(
                            qt_t[:],
                            qt_d[:, mc * 512:(mc + 1) * 512].rearrange(
                                "(dj pp) m -> pp dj m", pp=128
                            ),
                        )
                        # K (and Q for the first half of columns): 2+2 psum groups
                        for mt in range(PAIRS):
                            kps = psA.tile([128, 512], F32, name=f"it{it}_kps_{p}_{mc}_{mt}",
                                           tag="acc")
                            for dj in range(8):
                                nc.tensor.matmul(
                                    kps[:],
                                    wk_t[:, dj, mt * 128:(mt + 1) * 128],
                                    qt_t[:, dj, :],
                                    start=(dj == 0), stop=(dj == 7),
                                )
                            nc.vector.tensor_copy(
                                kht_t[:, mt, mc * 512:(mc + 1) * 512], kps[:]
                            )
                        if mc < HALF // 512:
                            for mt in range(PAIRS):
                                qps = psA.tile([128, 512], F32, name=f"it{it}_qps_{p}_{mc}_{mt}",
                                               tag="acc")
                                for dj in range(8):
                                    nc.tensor.matmul(
                                        qps[:],
                                        wq_t[:, dj, mt * 128:(mt + 1) * 128],
                                        qt_t[:, dj, :],
                                        start=(dj == 0), stop=(dj == 7),
                                    )
                                nc.vector.tensor_copy(
                                    qht_t[:, mt, mc * 512:(mc + 1) * 512], qps[:]
                                )
                        # V: 4 m-subtiles of 128, N = HP*65 = 260
                        for ms in range(4):
                            mi = mc * 4 + ms
                            vps = psA.tile([128, HP * 65], F32, name=f"it{it}_vps_{p}_{mi}",
                                           tag="acc")
                            for dj in range(8):
                                nc.tensor.matmul(
                                    vps[:],
                                    qt_t[:, dj, ms * 128:(ms + 1) * 128],
                                    wv_t[:, dj, :],
                                    start=(dj == 0), stop=(dj == 7),
                                )
                            nc.vector.tensor_copy(vaug_t[:, mi, :], vps[:])
                    # ones columns of V_aug
                    for hl in range(HP):
                        nc.vector.tensor_copy(
                            vaug_t[:, :, hl * 65 + 64], ones_sb[:, 0:MT]
                        )

                    # ---- attention ----
                    for a in range(PAIRS):
                        cj = p * PAIRS + a
                        for qc in range(HALF // 512):
                            qs = slice(qc * 512, (qc + 1) * 512)
                            ot = {}
                            for par in range(2):
                                ot[par] = psA.tile([65, 512], F32,
                                                   name=f"it{it}_ot_{p}_{a}_{qc}_{par}",
                                                   tag="acc")
                            for mi in range(MT):
                                ms_ = slice(mi * 128, (mi + 1) * 128)
                                sp = psS.tile([128, 1024], F32,
                                              name=f"it{it}_s_{p}_{a}_{qc}_{mi}",
                                              tag="score")
                                for par in range(2):
                                    nc.tensor.matmul(
                                        sp[:, 512 * par:512 * (par + 1)],
                                        kht_t[64 * par:64 * (par + 1), a, ms_],
                                        qht_t[64 * par:64 * (par + 1), a, qs],
                                        start=True, stop=True,
                                        tile_position=(64 * par, 0),
                                    )
                                ex = p2.tile([128, 1024], F32R,
                                             name=f"it{it}_e_{p}_{a}_{qc}_{mi}",
                                             tag="exp")
                                nc.scalar.activation(ex[:], sp[:], AF.Exp,
                                                     scale=1.0 / TEMPER)
                                for par in range(2):
                                    hl = 2 * a + par
                                    nc.tensor.matmul(
                                        ot[par][:],
                                        vaug_t[:, mi, hl * 65:(hl + 1) * 65],
                                        ex[:, 512 * par:512 * (par + 1)],
                                        start=(mi == 0), stop=(mi == MT - 1),
                                    )
                            for par in range(2):
                                den = p1.tile([128, 512], F32R,
                                              name=f"it{it}_den_{p}_{a}_{qc}_{par}", tag="den")
                                nc.vector.tensor_copy(den[64:65, :], ot[par][64:65, :])
                                bc = psA.tile([64, 512], F32,
                                              name=f"it{it}_bc_{p}_{a}_{qc}_{par}", tag="acc")
                                nc.tensor.matmul(bc[:], ones_t[64:65, :],
                                                 den[64:65, :], start=True, stop=True)
                                rec = p1.tile([64, 512], F32,
                                              name=f"it{it}_rec_{p}_{a}_{qc}_{par}", tag="rec")
                                nc.vector.reciprocal_approx_fast(rec[:], bc[:])
                                if par == 0:
                                    nc.vector.tensor_mul(
                                        ct_t[0:64, cj, qs], ot[par][0:64, :], rec[:]
                                    )
                                else:
                                    stg = p1.tile([64, 512], F32R,
                                                  name=f"it{it}_stg_{p}_{a}_{qc}", tag="stg")
                                    nc.vector.tensor_mul(stg[:], ot[par][0:64, :], rec[:])
                                    nc.sync.dma_start(ct_t[64:128, cj, qs], stg[:])

                # ---- output projection + residual + layernorm ----
                pw_t = {}
                for oc in range(2):
                    pw_t[oc] = p2.tile([128, 8, 512], F32R,
                                       name=f"it{it}_pwt_{oc}", tag="kht")
                    nc.sync.dma_start(
                        pw_t[oc][:],
                        pw_d[:, oc * 512:(oc + 1) * 512].rearrange(
                            "(dj pp) f -> pp dj f", pp=128
                        ),
                    )
                for qtb in range(4):
                    yts = {}
                    for qt in range(2):
                        yts[qt] = p2.tile([128, D], F32,
                                          name=f"it{it}_yt_{qtb}_{qt}", tag="y")
                        qti0 = qtb * 2 + qt
                        nc.sync.dma_start(
                            yts[qt][:], qres_d[qti0 * 128:(qti0 + 1) * 128, :])
                    for oc in range(2):
                        ypss = {}
                        for cjj in range(H // 2):
                            for qt in range(2):
                                qti = qtb * 2 + qt
                                if cjj == 0:
                                    ypss[qt] = psA.tile(
                                        [128, 512], F32,
                                        name=f"it{it}_y_{qtb}_{oc}_{qt}", tag="acc")
                                nc.tensor.matmul(
                                    ypss[qt][:],
                                    ct_t[:, cjj, qti * 128:(qti + 1) * 128],
                                    pw_t[oc][:, cjj, :],
                                    start=(cjj == 0), stop=(cjj == H // 2 - 1),
                                )
                        for qt in range(2):
                            nc.vector.tensor_add(
                                yts[qt][:, oc * 512:(oc + 1) * 512],
                                yts[qt][:, oc * 512:(oc + 1) * 512],
                                ypss[qt][:],
                            )
                    for qt in range(2):
                        qti = qtb * 2 + qt
                        y_t = yts[qt]
                        # layernorm: mu, sigma (ddof=1), (y-mu)/(sigma+eps)*a+b
                        s = p1.tile([128, 1], F32, name=f"it{it}_s_{qti}", tag="ln_s")
                        nc.vector.reduce_sum(s[:], y_t[:], axis=AX.X)
                        negmean = p1.tile([128, 1], F32, name=f"it{it}_nm_{qti}", tag="ln_nm")
                        nc.vector.tensor_scalar_mul(negmean[:], s[:], -1.0 / D)
                        mean = p1.tile([128, 1], F32, name=f"it{it}_m_{qti}", tag="ln_m")
                        nc.vector.tensor_scalar_mul(mean[:], s[:], 1.0 / D)
                        ss = p1.tile([128, 1], F32, name=f"it{it}_ss_{qti}", tag="ln_ss")
                        ss2 = p1.tile([128, 1], F32, name=f"it{it}_ss2_{qti}", tag="ln_ss2")
                        for oc in range(2):
                            sq = psS.tile([128, 512], F32, name=f"it{it}_sq_{qti}_{oc}",
                                          tag="score")
                            nc.scalar.activation(
                                sq[:], y_t[:, oc * 512:(oc + 1) * 512], AF.Square,
                                bias=negmean[:],
                                accum_out=(ss[:] if oc == 0 else ss2[:]),
                            )
                        nc.vector.tensor_add(ss[:], ss[:], ss2[:])
                        sigma = p1.tile([128, 1], F32, name=f"it{it}_sg_{qti}", tag="ln_sg")
                        nc.scalar.activation(sigma[:], ss[:], AF.Sqrt,
                                             scale=1.0 / (D - 1))
                        var = p1.tile([128, 1], F32, name=f"it{it}_var_{qti}", tag="ln_var")
                        nc.vector.tensor_scalar_mul(var[:], ss[:], 1.0 / (D - 1))
                        rs = p1.tile([128, 1], F32, name=f"it{it}_rs_{qti}", tag="ln_rs")
                        nc.vector.reciprocal(rs[:], sigma[:])
                        t1 = p1.tile([128, 1], F32, name=f"it{it}_t1_{qti}", tag="ln_t1")
                        nc.vector.tensor_mul(t1[:], var[:], rs[:])
                        nc.vector.tensor_add(t1[:], t1[:], sigma[:])
                        dd = p1.tile([128, 1], F32, name=f"it{it}_dd_{qti}", tag="ln_dd")
                        nc.vector.tensor_scalar(dd[:], t1[:], 0.5, LN_EPS,
                                                ALU.mult, ALU.add)
                        rec2 = p1.tile([128, 1], F32, name=f"it{it}_rc_{qti}", tag="ln_rc")
                        nc.vector.reciprocal(rec2[:], dd[:])
                        o_t = p2.tile([128, D], F32, name=f"it{it}_o_{qti}", tag="o")
                        nc.vector.tensor_scalar(o_t[:], y_t[:], mean[:], rec2[:],
                                                ALU.subtract, ALU.mult)
                        nc.vector.tensor_mul(o_t[:], o_t[:], lna_t[:])
                        nc.vector.tensor_add(o_t[:], o_t[:], lnb_t[:])
                        nc.sync.dma_start(out_d[qti * 128:(qti + 1) * 128, :], o_t[:])

    nc.compile()
    return nc


def _get_nc():
    if "nc" not in _CACHE:
        _CACHE["nc"] = build()
    return _CACHE["nc"]


def kernel(q, w_qs, w_ks, w_vs, proj_w, proj_b, ln_a, ln_b, **kw):
    q = np.asarray(q, dtype=np.float32)
    w_qs = np.asarray(w_qs, dtype=np.float32)
    w_ks = np.asarray(w_ks, dtype=np.float32)
    w_vs = np.asarray(w_vs, dtype=np.float32)
    proj_w = np.asarray(proj_w, dtype=np.float32)
    proj_b = np.asarray(proj_b, dtype=np.float32)
    ln_a = np.asarray(ln_a, dtype=np.float32)
    ln_b = np.asarray(ln_b, dtype=np.float32)

    wq_all = np.ascontiguousarray(w_qs.transpose(1, 0, 2).reshape(D, H * DK))
    wk_all = np.ascontiguousarray(w_ks.transpose(1, 0, 2).reshape(D, H * DK))
    wv_aug = np.zeros((D, H, 65), dtype=np.float32)
    wv_aug[:, :, :64] = w_vs.transpose(1, 0, 2)
    wv_aug = np.ascontiguousarray(wv_aug.reshape(D, H * 65))
    pwT = np.ascontiguousarray(proj_w.T)
    ones64 = np.ones((1, 64), dtype=np.float32)
    lna = np.ascontiguousarray(ln_a[None, :])
    lnb = np.ascontiguousarray(ln_b[None, :])

    in_maps = []
    for c in range(8):
        b, half = c // 2, c % 2
        qbT = q[b].T  # [D, L]
        qt_c = np.ascontiguousarray(
            np.concatenate(
                [qbT[:, half * HALF:(half + 1) * HALF],
                 qbT[:, (1 - half) * HALF:(2 - half) * HALF]],
                axis=1,
            )
        )
        qres_c = np.ascontiguousarray(
            q[b, half * HALF:(half + 1) * HALF, :] + proj_b[None, :]
        )
        in_maps.append({
            "qt": qt_c, "qres": qres_c,
            "wq": wq_all, "wk": wk_all, "wv": wv_aug, "pw": pwT,
            "lna": lna, "lnb": lnb, "ones64": ones64,
        })

    nc = _get_nc()
    res = run_bass_kernel_spmd(nc, in_maps, core_ids=list(range(8))).results

    out = np.empty((B, L, D), dtype=np.float32)
    for c in range(8):
        b, half = c // 2, c % 2
        out[b, half * HALF:(half + 1) * HALF, :] = res[c]["out"]
    return out



# revision 8
# speedup vs baseline: 1.7398x; 1.7398x over previous
"""MultiHeadAttention TRN2 Bass kernel (8 NeuronCores), fp8 DoubleRow edition.

Sharding: core c = (batch b = c//2, query-half = c%2). Each core computes
K/V for its full batch (2048 keys) and attention + output projection + LN
for its 1024 query rows. No collectives; host gathers per-core outputs.

All heavy matmuls run in fp8e4m3 with MatmulPerfMode.DoubleRow (0.5 PE
cycles per output row, 256-wide contraction per instruction):
  - Q/K projections:  out[4head*32dk half, q|m] over D=1024 (4 DR chunks)
  - V projection:     Vaug[m, 4head*65] (col 0 of each head = ones so the
                      softmax denominator lands at partition 0 of OT)
  - scores:           S[m, q] per head, dk=64 = 2x32 DR slices at
                      tile_position (32*hh, 0)
  - attn*V:           OT[65, q] accumulated over 8 DR m-pair chunks
  - output proj:      Y[q, o] over 8 chunks of [65,2] (denominator row is
                      multiplied by zeroed pw rows)

PSUM can only be read by ACT and DVE, so exp(S/32), the K/Q/V fp8
converts, OT drains, and residual adds are greedily load-balanced
between those two engines at build time; Pool (gpsimd) handles all
SBUF-side work (denominator broadcast + CT scale, LN stats + finals).
exp on DVE uses the int8 bit trick:
  i8 = rne(S*8/(32*ln2) + 55.63), bitcast int8 -> fp8e4m3 ~= exp(S/32)
(the denominator sums the same approximated values -> consistent).

LayerNorm: device computes z = (y - mu)/(sigma_ddof1 + eps); the ln_a/ln_b
affine is applied on host. V is scaled x8 on host (pw /8) to keep CT
inside the fp8 normal range.
"""
import numpy as np
import ml_dtypes

import concourse.bass as bass
import concourse.mybir as mybir
import concourse.tile as tile
from concourse import bacc
from concourse.bass_utils import run_bass_kernel_spmd

F32 = mybir.dt.float32
F32R = mybir.dt.float32r
F8 = mybir.dt.float8e4
I8 = mybir.dt.int8
AF = mybir.ActivationFunctionType
ALU = mybir.AluOpType
PM = mybir.MatmulPerfMode
E4M3 = ml_dtypes.float8_e4m3

B, L, D = 4, 2048, 1024
H, DK = 16, 64
HALF = 1024            # query rows per core
TEMPER = 32.0          # sqrt(d_model)
G = 4                  # head groups of 4
LN_EPS = 1e-3
VSCALE = 8.0           # host scales w_vs by this, pw by 1/this
EXP_S1 = float(8.0 / (TEMPER * np.log(2.0)))
EXP_S2 = 56.0 - 0.37   # rne magic (calibrated on hw)

_CACHE = {}


def build(iters=1):
    nc = bacc.Bacc(None, target_bir_lowering=False)
    qt8_d = nc.dram_tensor("qt8", [128, G * 2 * L], F8, kind="ExternalInput")
    wq8_d = nc.dram_tensor("wq8", [128, G * 2 * 1024], F8, kind="ExternalInput")
    wk8_d = nc.dram_tensor("wk8", [128, G * 2 * 1024], F8, kind="ExternalInput")
    wv8_d = nc.dram_tensor("wv8", [128, G * 2 * 1040], F8, kind="ExternalInput")
    pw8_d = nc.dram_tensor("pw8", [65, 8 * 2 * 1024], F8, kind="ExternalInput")
    qres_d = nc.dram_tensor("qres", [HALF, D], F32, kind="ExternalInput")
    out_d = nc.dram_tensor("out", [HALF, D], F32, kind="ExternalOutput")

    # build-time greedy ACT/DVE balancing (ns estimates incl. seq overhead)
    eng_ns = {"A": 0.0, "D": 0.0}

    def pick_ad(rows):
        ca = rows * 0.853 + 57.0
        cd = rows * 1.065 + 70.0
        if eng_ns["A"] + ca <= eng_ns["D"] + cd:
            eng_ns["A"] += ca
            return "A"
        eng_ns["D"] += cd
        return "D"

    def charge_d(rows):
        eng_ns["D"] += rows * 1.065 + 70.0

    def ad_copy(e, dst, src):
        if e == "A":
            nc.scalar.activation(dst, src, AF.Copy)
        else:
            nc.vector.tensor_copy(dst, src)

    with tile.TileContext(nc) as tc:
        with (
            tc.tile_pool(name="p1", bufs=1) as p1,
            tc.tile_pool(name="p2", bufs=2) as p2,
            tc.tile_pool(name="p3", bufs=3) as p3,
            tc.tile_pool(name="psS", bufs=2, space="PSUM") as psS,
            tc.tile_pool(name="psO", bufs=2, space="PSUM") as psO,
            tc.tile_pool(name="psA", bufs=2, space="PSUM") as psA,
        ):
            # ---- weight loads ----
            wk8_t = p1.tile([128, G, 2, 1024], F8, name="wk8_t")
            nc.sync.dma_start(wk8_t[:], wk8_d[:])
            qt8_t = p1.tile([128, G, 2, L], F8, name="qt8_t")
            for j in range(G):
                nc.sync.dma_start(qt8_t[:, j, :, :],
                                  qt8_d[:, j * 2 * L:(j + 1) * 2 * L])
            wq8_t = p1.tile([128, G, 2, 1024], F8, name="wq8_t")
            nc.sync.dma_start(wq8_t[:], wq8_d[:])
            wv8_t = p1.tile([128, G, 2, 1040], F8, name="wv8_t")
            nc.sync.dma_start(wv8_t[:], wv8_d[:])
            pw8_t = p1.tile([65, 8, 2, 1024], F8, name="pw8_t")
            nc.sync.dma_start(pw8_t[:], pw8_d[:])

            for it in range(iters):
                sfx = f"i{it}"
                # per-group fp8 activation stores
                q8 = p1.tile([128, G, 2, HALF], F8, name=f"q8_{sfx}")
                k8 = p1.tile([128, G, 2, L], F8, name=f"k8_{sfx}")
                v8 = p1.tile([128, G, 8, 2, 320], F8, name=f"v8_{sfx}")
                ct8 = p1.tile([65, 8, 2, HALF], F8, name=f"ct8_{sfx}")
                # ones columns of v8 (col 0 of each head's 65-block);
                # the V convert copies only fill cols 1..64.
                for hh in range(4):
                    nc.gpsimd.memset(v8[:, :, :, :, hh * 80], 1.0)

                for g in range(G):
                    # ==== projections for group g ====
                    for s in range(2):
                        for mb in range(4):
                            kp = psA.tile([128, 512], F32,
                                          name=f"kp_{sfx}_{g}_{s}_{mb}", tag="acc")
                            for j in range(G):
                                nc.tensor.matmul(
                                    kp[:],
                                    wk8_t[:, j, :, g * 256 + s * 128:
                                          g * 256 + s * 128 + 128],
                                    qt8_t[:, j, :, mb * 512:(mb + 1) * 512],
                                    start=(j == 0), stop=(j == G - 1),
                                    perf_mode=PM.DoubleRow,
                                )
                            ad_copy(pick_ad(512),
                                    k8[:, g, s, mb * 512:(mb + 1) * 512], kp[:])
                    for s in range(2):
                        for qb in range(2):
                            qp = psA.tile([128, 512], F32,
                                          name=f"qp_{sfx}_{g}_{s}_{qb}", tag="acc")
                            for j in range(G):
                                nc.tensor.matmul(
                                    qp[:],
                                    wq8_t[:, j, :, g * 256 + s * 128:
                                          g * 256 + s * 128 + 128],
                                    qt8_t[:, j, :, qb * 512:(qb + 1) * 512],
                                    start=(j == 0), stop=(j == G - 1),
                                    perf_mode=PM.DoubleRow,
                                )
                            ad_copy(pick_ad(512),
                                    q8[:, g, s, qb * 512:(qb + 1) * 512], qp[:])
                    for mt in range(16):
                        vp = psO.tile([128, 512], F32,
                                      name=f"vp_{sfx}_{g}_{mt}", tag="ot")
                        for j in range(G):
                            nc.tensor.matmul(
                                vp[:, 0:260],
                                qt8_t[:, j, :, mt * 128:(mt + 1) * 128],
                                wv8_t[:, j, :, g * 260:(g + 1) * 260],
                                start=(j == 0), stop=(j == G - 1),
                                perf_mode=PM.DoubleRow,
                            )
                        ad_copy(
                            pick_ad(256),
                            v8[:, g, mt // 2, mt % 2, :]
                            .rearrange("p (h f) -> p h f", h=4)[:, :, 1:65],
                            vp[:, 0:260]
                            .rearrange("p (h f) -> p h f", h=4)[:, :, 1:65],
                        )

                    # ==== attention for group g ====
                    for qc in range(2):
                        qs = slice(qc * 512, (qc + 1) * 512)
                        for hh in range(4):
                            p0 = 32 * hh
                            prow = slice(p0, p0 + 32)
                            ot = psO.tile([128, 512], F32,
                                          name=f"ot_{sfx}_{g}_{qc}_{hh}", tag="ot")
                            for mip in range(8):
                                sp = psS.tile([128, 1024], F32,
                                              name=f"sp_{sfx}_{g}_{qc}_{hh}_{mip}",
                                              tag="sc")
                                for k in range(2):
                                    mi = 2 * mip + k
                                    nc.tensor.matmul(
                                        sp[:, k * 512:(k + 1) * 512],
                                        k8[prow, g, :, mi * 128:(mi + 1) * 128],
                                        q8[prow, g, :, qs],
                                        start=True, stop=True,
                                        perf_mode=PM.DoubleRow,
                                        tile_position=(p0, 0),
                                    )
                                e8 = p3.tile([128, 2, 512], F8,
                                             name=f"e8_{sfx}_{g}_{qc}_{hh}_{mip}",
                                             tag="e8")
                                if pick_ad(1024) == "A":
                                    nc.scalar.activation(
                                        e8[:].rearrange("p s f -> p (s f)"),
                                        sp[:], AF.Exp, scale=1.0 / TEMPER)
                                else:
                                    nc.vector.tensor_scalar(
                                        e8[:].bitcast(I8).rearrange(
                                            "p s f -> p (s f)"),
                                        sp[:], EXP_S1, EXP_S2,
                                        ALU.mult, ALU.add)
                                nc.tensor.matmul(
                                    ot[0:65, :],
                                    v8[:, g, mip, :, hh * 80:hh * 80 + 65],
                                    e8[:],
                                    start=(mip == 0), stop=(mip == 7),
                                    perf_mode=PM.DoubleRow,
                                )
                            # drain -> recip(denominator) -> bcast -> scale
                            stage = p2.tile([65, 512], F32,
                                            name=f"st_{sfx}_{g}_{qc}_{hh}",
                                            tag="otst")
                            ad_copy(pick_ad(512), stage[:], ot[0:65, :])
                            rc = p2.tile([1, 512], F32,
                                         name=f"rc_{sfx}_{g}_{qc}_{hh}", tag="rc")
                            nc.vector.reciprocal_approx_fast(rc[:], stage[0:1, :])
                            charge_d(512)
                            rcb = p2.tile([65, 512], F32,
                                          name=f"rcb_{sfx}_{g}_{qc}_{hh}",
                                          tag="rcb")
                            nc.gpsimd.partition_broadcast(rcb[:], rc[:])
                            h = 4 * g + hh
                            nc.gpsimd.tensor_tensor(
                                ct8[:, h // 2, h % 2, qs],
                                stage[:], rcb[:], ALU.mult)

                # ======== output projection + layernorm ========
                sums = p1.tile([128, 16], F32, name=f"sums_{sfx}")
                ssq16 = p1.tile([128, 16], F32, name=f"ssq16_{sfx}")
                mu8 = p1.tile([128, 8], F32, name=f"mu8_{sfx}")
                m28 = p1.tile([128, 8], F32, name=f"m28_{sfx}")
                ssq8 = p1.tile([128, 8], F32, name=f"ssq8_{sfx}")
                cs8 = p1.tile([128, 8], F32, name=f"cs8_{sfx}")
                rs8 = p1.tile([128, 8], F32, name=f"rs8_{sfx}")
                rr8 = p1.tile([128, 8], F32, name=f"rr8_{sfx}")
                rec8 = p1.tile([128, 8], F32, name=f"rec8_{sfx}")
                y_ts = []
                for qt in range(8):
                    y_t = p1.tile([128, D], F32, name=f"y_{sfx}_{qt}")
                    y_ts.append(y_t)
                    qres_t = p2.tile([128, D], F32, name=f"qr_{sfx}_{qt}", tag="qr")
                    nc.sync.dma_start(qres_t[:],
                                      qres_d[qt * 128:(qt + 1) * 128, :])
                    for oc in range(2):
                        yp = psA.tile([128, 512], F32,
                                      name=f"yp_{sfx}_{qt}_{oc}", tag="acc")
                        for j in range(8):
                            nc.tensor.matmul(
                                yp[:],
                                ct8[:, j, :, qt * 128:(qt + 1) * 128],
                                pw8_t[:, j, :, oc * 512:(oc + 1) * 512],
                                start=(j == 0), stop=(j == 7),
                                perf_mode=PM.DoubleRow,
                            )
                        nc.vector.scalar_tensor_tensor(
                            y_t[:, oc * 512:(oc + 1) * 512],
                            yp[:], 1.0, qres_t[:, oc * 512:(oc + 1) * 512],
                            ALU.mult, ALU.add,
                            accum_out=sums[:, 2 * qt + oc:2 * qt + oc + 1])
                        charge_d(512)
                    # sum of squares via ACT Square + accumulator
                    sqt = p2.tile([128, D], F32, name=f"sqt_{sfx}_{qt}",
                                  tag="sqt")
                    for oc in range(2):
                        nc.scalar.activation(
                            sqt[:, oc * 512:(oc + 1) * 512],
                            y_t[:, oc * 512:(oc + 1) * 512], AF.Square,
                            accum_out=ssq16[:, 2 * qt + oc:2 * qt + oc + 1])
                        eng_ns["A"] += 512 * 0.853 + 57.0
                    nc.gpsimd.tensor_tensor(mu8[:, qt:qt + 1],
                                            sums[:, 2 * qt:2 * qt + 1],
                                            sums[:, 2 * qt + 1:2 * qt + 2],
                                            ALU.add)
                # batched sigma chain over all 8 tiles
                nc.gpsimd.tensor_tensor(ssq8[:], ssq16[:, 0::2],
                                        ssq16[:, 1::2], ALU.add)
                nc.gpsimd.tensor_scalar(mu8[:], mu8[:], 1.0 / D, None, ALU.mult)
                nc.gpsimd.tensor_tensor(m28[:], mu8[:], mu8[:], ALU.mult)
                nc.vector.scalar_tensor_tensor(cs8[:], m28[:], -float(D),
                                               ssq8[:], ALU.mult, ALU.add)
                nc.scalar.activation(rs8[:], cs8[:], AF.Abs_reciprocal_sqrt,
                                     scale=1.0 / ((D - 1) * VSCALE * VSCALE))
                nc.gpsimd.tensor_tensor(rr8[:], rs8[:], rs8[:], ALU.mult)
                nc.vector.scalar_tensor_tensor(rec8[:], rr8[:], -LN_EPS,
                                               rs8[:], ALU.mult, ALU.add)
                for qt in range(8):
                    o_t = p2.tile([128, D], F32, name=f"o_{sfx}_{qt}", tag="o")
                    nc.gpsimd.tensor_scalar(o_t[:], y_ts[qt][:],
                                            mu8[:, qt:qt + 1],
                                            rec8[:, qt:qt + 1],
                                            ALU.subtract, ALU.mult)
                    nc.sync.dma_start(out_d[qt * 128:(qt + 1) * 128, :], o_t[:])

    nc.compile()
    return nc


def _get_nc():
    if "nc" not in _CACHE:
        _CACHE["nc"] = build()
    return _CACHE["nc"]


def _prep_shared(w_qs, w_ks, w_vs, proj_w):
    """fp8 weight layouts: rows d -> [p, j, s] with d = 256j + 128s + p."""
    def dsplit(a):  # [1024, N] -> [128, 4*2*N]
        n = a.shape[1]
        return np.ascontiguousarray(
            a.reshape(G, 2, 128, n).transpose(2, 0, 1, 3).reshape(128, -1)
        )

    # wq/wk cols: g*256 + (dk//32)*128 + hh*32 + dk%32  <- head 4g+hh
    wq = np.empty((D, H * DK), dtype=np.float32)
    wk = np.empty((D, H * DK), dtype=np.float32)
    for g in range(G):
        for s in range(2):
            for hh in range(4):
                c0 = g * 256 + s * 128 + hh * 32
                wq[:, c0:c0 + 32] = w_qs[4 * g + hh, :, 32 * s:32 * s + 32]
                wk[:, c0:c0 + 32] = w_ks[4 * g + hh, :, 32 * s:32 * s + 32]
    # wv cols: g*260 + hh*65 + (1+dv); col hh*65 is the ones slot
    wv = np.zeros((D, G * 4 * 65), dtype=np.float32)
    for g in range(G):
        for hh in range(4):
            c0 = g * 260 + hh * 65
            wv[:, c0 + 1:c0 + 65] = w_vs[4 * g + hh] * VSCALE
    # pw8 [65, 8, 2, 1024]: row p=0 zero (denominator slot), p=1+dv maps
    # to concat row (2j+s)*64+dv of proj_w.T
    pwT = proj_w.T.astype(np.float32)  # [c, o]
    pw8 = np.zeros((65, 8, 2, D), dtype=np.float32)
    for j in range(8):
        for s in range(2):
            h = 2 * j + s
            pw8[1:65, j, s, :] = pwT[h * 64:(h + 1) * 64, :]
    pw8 = pw8.reshape(65, -1)
    wq8 = dsplit(wq).astype(E4M3)
    wk8 = dsplit(wk).astype(E4M3)
    wv8 = dsplit(wv).astype(E4M3)
    pw8 = np.ascontiguousarray(pw8).astype(E4M3)
    return wq8, wk8, wv8, pw8


def kernel(q, w_qs, w_ks, w_vs, proj_w, proj_b, ln_a, ln_b, **kw):
    q = np.asarray(q, dtype=np.float32)
    w_qs = np.asarray(w_qs, dtype=np.float32)
    w_ks = np.asarray(w_ks, dtype=np.float32)
    w_vs = np.asarray(w_vs, dtype=np.float32)
    proj_w = np.asarray(proj_w, dtype=np.float32)
    proj_b = np.asarray(proj_b, dtype=np.float32)
    ln_a = np.asarray(ln_a, dtype=np.float32)
    ln_b = np.asarray(ln_b, dtype=np.float32)

    wq8, wk8, wv8, pw8 = _prep_shared(w_qs, w_ks, w_vs, proj_w)

    in_maps = []
    for c in range(8):
        b, half = c // 2, c % 2
        qbT = q[b].T  # [D, L]
        qcat = np.concatenate(
            [qbT[:, half * HALF:(half + 1) * HALF],
             qbT[:, (1 - half) * HALF:(2 - half) * HALF]], axis=1)
        qt8 = np.ascontiguousarray(
            qcat.reshape(G, 2, 128, L).transpose(2, 0, 1, 3).reshape(128, -1)
        ).astype(E4M3)
        qres_c = np.ascontiguousarray(
            (q[b, half * HALF:(half + 1) * HALF, :] + proj_b[None, :]) * VSCALE)
        in_maps.append({
            "qt8": qt8, "qres": qres_c,
            "wq8": wq8, "wk8": wk8, "wv8": wv8, "pw8": pw8,
        })

    nc = _get_nc()
    res = run_bass_kernel_spmd(nc, in_maps, core_ids=list(range(8))).results

    out = np.empty((B, L, D), dtype=np.float32)
    for c in range(8):
        b, half = c // 2, c % 2
        out[b, half * HALF:(half + 1) * HALF, :] = res[c]["out"]
    # ln affine on host
    out = out * (ln_a[None, None, :] / VSCALE) + ln_b[None, None, :]
    return out


# revision 9
# speedup vs baseline: 1.8201x; 1.0462x over previous
"""MultiHeadAttention TRN2 Bass kernel (8 NeuronCores), fp8 DoubleRow edition.

Sharding: core c = (batch b = c//2, query-half = c%2). Each core computes
K/V for its full batch (2048 keys) and attention + output projection + LN
for its 1024 query rows. No collectives; host gathers per-core outputs.

All heavy matmuls run in fp8e4m3 with MatmulPerfMode.DoubleRow (0.5 PE
cycles per output row, 256-wide contraction per instruction):
  - Q/K projections:  out[4head*32dk half, q|m] over D=1024 (4 DR chunks)
  - V projection:     Vaug[m, 4head*65] (col 0 of each head = ones so the
                      softmax denominator lands at partition 0 of OT)
  - scores:           S[m, q] per head, dk=64 = 2x32 DR slices at
                      tile_position (32*hh, 0)
  - attn*V:           OT[65, q] accumulated over 8 DR m-pair chunks
  - output proj:      Y[q, o] over 8 chunks of [65,2] (denominator row is
                      multiplied by zeroed pw rows)

PSUM can only be read by ACT and DVE, so exp(S/32), the K/Q/V fp8
converts, OT drains, and residual adds are greedily load-balanced
between those two engines at build time; Pool (gpsimd) handles all
SBUF-side work (denominator broadcast + CT scale, LN stats + finals).
exp on DVE uses the int8 bit trick:
  i8 = rne(S*8/(32*ln2) + 55.63), bitcast int8 -> fp8e4m3 ~= exp(S/32)
(the denominator sums the same approximated values -> consistent).

LayerNorm: device computes z = (y - mu)/(sigma_ddof1 + eps); the ln_a/ln_b
affine is applied on host. V is scaled x8 on host (pw /8) to keep CT
inside the fp8 normal range.
"""
import numpy as np
import ml_dtypes

import concourse.bass as bass
import concourse.mybir as mybir
import concourse.tile as tile
from concourse import bacc
from concourse.bass_utils import run_bass_kernel_spmd

F32 = mybir.dt.float32
F32R = mybir.dt.float32r
F8 = mybir.dt.float8e4
I8 = mybir.dt.int8
AF = mybir.ActivationFunctionType
ALU = mybir.AluOpType
PM = mybir.MatmulPerfMode
E4M3 = ml_dtypes.float8_e4m3

B, L, D = 4, 2048, 1024
H, DK = 16, 64
HALF = 1024            # query rows per core
TEMPER = 32.0          # sqrt(d_model)
G = 4                  # head groups of 4
LN_EPS = 1e-3
VSCALE = 8.0           # host scales w_vs by this, pw by 1/this
EXP_S1 = float(8.0 / (TEMPER * np.log(2.0)))
EXP_S2 = 56.0 - 0.37   # rne magic (calibrated on hw)

_CACHE = {}


def build(iters=1):
    nc = bacc.Bacc(None, target_bir_lowering=False)
    qt8_d = nc.dram_tensor("qt8", [128, G * 2 * L], F8, kind="ExternalInput")
    wq8_d = nc.dram_tensor("wq8", [128, G * 2 * 1024], F8, kind="ExternalInput")
    wk8_d = nc.dram_tensor("wk8", [128, G * 2 * 1024], F8, kind="ExternalInput")
    wv8_d = nc.dram_tensor("wv8", [128, G * 2 * 1040], F8, kind="ExternalInput")
    pw8_d = nc.dram_tensor("pw8", [65, 8 * 2 * 1024], F8, kind="ExternalInput")
    qres_d = nc.dram_tensor("qres", [HALF, D], F32, kind="ExternalInput")
    out_d = nc.dram_tensor("out", [HALF, D], F32, kind="ExternalOutput")

    # build-time greedy ACT/DVE balancing (ns estimates incl. seq overhead)
    eng_ns = {"A": 0.0, "D": 0.0}

    def pick_ad(rows):
        ca = rows * 0.853 + 124.0
        cd = rows * 1.065 + 108.0
        if eng_ns["A"] + ca <= eng_ns["D"] + cd:
            eng_ns["A"] += ca
            return "A"
        eng_ns["D"] += cd
        return "D"

    def charge_d(rows):
        eng_ns["D"] += rows * 1.065 + 108.0

    def ad_copy(e, dst, src):
        if e == "A":
            nc.scalar.activation(dst, src, AF.Copy)
        else:
            nc.vector.tensor_copy(dst, src)

    with tile.TileContext(nc) as tc:
        with (
            tc.tile_pool(name="p1", bufs=1) as p1,
            tc.tile_pool(name="p2", bufs=2) as p2,
            tc.tile_pool(name="p3", bufs=4) as p3,
            tc.tile_pool(name="psS", bufs=2, space="PSUM") as psS,
            tc.tile_pool(name="psO", bufs=2, space="PSUM") as psO,
            tc.tile_pool(name="psA", bufs=2, space="PSUM") as psA,
        ):
            # ---- weight loads ----
            wk8_t = p1.tile([128, G, 2, 1024], F8, name="wk8_t")
            nc.sync.dma_start(wk8_t[:], wk8_d[:])
            qt8_t = p1.tile([128, G, 2, L], F8, name="qt8_t")
            for j in range(G):
                nc.sync.dma_start(qt8_t[:, j, :, :],
                                  qt8_d[:, j * 2 * L:(j + 1) * 2 * L])
            wq8_t = p1.tile([128, G, 2, 1024], F8, name="wq8_t")
            nc.sync.dma_start(wq8_t[:], wq8_d[:])
            wv8_t = p1.tile([128, G, 2, 1040], F8, name="wv8_t")
            nc.sync.dma_start(wv8_t[:], wv8_d[:])
            pw8_t = p1.tile([65, 8, 2, 1024], F8, name="pw8_t")
            nc.sync.dma_start(pw8_t[:], pw8_d[:])

            for it in range(iters):
                sfx = f"i{it}"
                # per-group fp8 activation stores
                q8 = p1.tile([128, G, 2, HALF], F8, name=f"q8_{sfx}")
                k8 = p1.tile([128, G, 2, L], F8, name=f"k8_{sfx}")
                v8 = p1.tile([128, G, 8, 2, 320], F8, name=f"v8_{sfx}")
                ct8 = p1.tile([65, 8, 2, HALF], F8, name=f"ct8_{sfx}")
                # ones columns of v8 (col 0 of each head's 80-block);
                # the V convert copies only fill cols 1..64.
                for hh in range(4):
                    nc.gpsimd.memset(v8[:, :, :, :, hh * 80], 1.0)

                # layernorm stat tiles
                sums = p1.tile([128, 16], F32, name=f"sums_{sfx}")
                ssq16 = p1.tile([128, 16], F32, name=f"ssq16_{sfx}")
                mu8 = p1.tile([128, 8], F32, name=f"mu8_{sfx}")
                m28 = p1.tile([128, 8], F32, name=f"m28_{sfx}")
                ssq8 = p1.tile([128, 8], F32, name=f"ssq8_{sfx}")
                cs8 = p1.tile([128, 8], F32, name=f"cs8_{sfx}")
                rs8 = p1.tile([128, 8], F32, name=f"rs8_{sfx}")
                rr8 = p1.tile([128, 8], F32, name=f"rr8_{sfx}")
                rec8 = p1.tile([128, 8], F32, name=f"rec8_{sfx}")
                y_ts = [p1.tile([128, D], F32, name=f"y_{sfx}_{qt}")
                        for qt in range(8)]

                # ---- deferred-emission item lists (PE filler machinery) ----
                def proj_items(g):
                    """K/Q/V projections for group g as single-instruction
                    closures (chunk matmuls + converts)."""
                    items = []

                    def kq_group(wt, dst, g, s, blk, nm):
                        hold = {}

                        def mm(j, hold=hold, g=g, s=s, blk=blk, nm=nm, wt=wt):
                            if j == 0:
                                hold["t"] = psA.tile(
                                    [128, 512], F32,
                                    name=f"{nm}_{sfx}_{g}_{s}_{blk}", tag="acc")
                            nc.tensor.matmul(
                                hold["t"][:],
                                wt[:, j, :, g * 256 + s * 128:
                                   g * 256 + s * 128 + 128],
                                qt8_t[:, j, :, blk * 512:(blk + 1) * 512],
                                start=(j == 0), stop=(j == G - 1),
                                perf_mode=PM.DoubleRow,
                            )

                        def cv(hold=hold, dst=dst):
                            ad_copy(pick_ad(512), dst, hold["t"][:])

                        return [lambda j=j: mm(j) for j in range(G)] + [cv]

                    for s in range(2):
                        for mb in range(4):
                            items += kq_group(
                                wk8_t, k8[:, g, s, mb * 512:(mb + 1) * 512],
                                g, s, mb, "kp")
                    for s in range(2):
                        for qb in range(2):
                            items += kq_group(
                                wq8_t, q8[:, g, s, qb * 512:(qb + 1) * 512],
                                g, s, qb, "qp")
                    for mt in range(16):
                        hold = {}

                        def vmm(j, hold=hold, g=g, mt=mt):
                            if j == 0:
                                hold["t"] = psA.tile(
                                    [128, 512], F32,
                                    name=f"vp_{sfx}_{g}_{mt}", tag="acc")
                            nc.tensor.matmul(
                                hold["t"][:, 0:260],
                                qt8_t[:, j, :, mt * 128:(mt + 1) * 128],
                                wv8_t[:, j, :, g * 260:(g + 1) * 260],
                                start=(j == 0), stop=(j == G - 1),
                                perf_mode=PM.DoubleRow,
                            )

                        def vcv(hold=hold, g=g, mt=mt):
                            ad_copy(
                                pick_ad(256),
                                v8[:, g, mt // 2, mt % 2, :]
                                .rearrange("p (h f) -> p h f", h=4)[:, :, 1:65],
                                hold["t"][:, 0:260]
                                .rearrange("p (h f) -> p h f", h=4)[:, :, 1:65],
                            )

                        items += [lambda j=j, f=vmm: f(j) for j in range(G)]
                        items.append(vcv)
                    return items

                def outproj_items(qts):
                    """Output projection + y-add + squares for given q tiles."""
                    items = []
                    for qt in qts:
                        qr_hold = {}

                        def qdma(qt=qt, h=qr_hold):
                            h["t"] = p2.tile([128, D], F32,
                                             name=f"qr_{sfx}_{qt}", tag="qr")
                            nc.sync.dma_start(
                                h["t"][:], qres_d[qt * 128:(qt + 1) * 128, :])

                        items.append(qdma)
                        for oc in range(2):
                            hold = {}

                            def ymm(j, hold=hold, qt=qt, oc=oc):
                                if j == 0:
                                    hold["t"] = psA.tile(
                                        [128, 512], F32,
                                        name=f"yp_{sfx}_{qt}_{oc}", tag="acc")
                                nc.tensor.matmul(
                                    hold["t"][:],
                                    ct8[:, j, :, qt * 128:(qt + 1) * 128],
                                    pw8_t[:, j, :, oc * 512:(oc + 1) * 512],
                                    start=(j == 0), stop=(j == 7),
                                    perf_mode=PM.DoubleRow,
                                )

                            def ystt(hold=hold, qt=qt, oc=oc, h=qr_hold):
                                nc.vector.scalar_tensor_tensor(
                                    y_ts[qt][:, oc * 512:(oc + 1) * 512],
                                    hold["t"][:], 1.0,
                                    h["t"][:, oc * 512:(oc + 1) * 512],
                                    ALU.mult, ALU.add,
                                    accum_out=sums[:, 2 * qt + oc:
                                                   2 * qt + oc + 1])
                                charge_d(512)

                            items += [lambda j=j, f=ymm: f(j) for j in range(8)]
                            items.append(ystt)

                        def sq(qt=qt):
                            sqt = p2.tile([128, D], F32,
                                          name=f"sqt_{sfx}_{qt}", tag="sqt")
                            for oc in range(2):
                                nc.scalar.activation(
                                    sqt[:, oc * 512:(oc + 1) * 512],
                                    y_ts[qt][:, oc * 512:(oc + 1) * 512],
                                    AF.Square,
                                    accum_out=ssq16[:, 2 * qt + oc:
                                                    2 * qt + oc + 1])
                                eng_ns["A"] += 512 * 0.853 + 124.0
                            nc.gpsimd.tensor_tensor(
                                mu8[:, qt:qt + 1],
                                sums[:, 2 * qt:2 * qt + 1],
                                sums[:, 2 * qt + 1:2 * qt + 2], ALU.add)

                        items.append(sq)
                    return items

                def emit_head(g, qc, hh, filler, pace):
                    qs = slice(qc * 512, (qc + 1) * 512)
                    p0 = 32 * hh
                    prow = slice(p0, p0 + 32)
                    ot = psO.tile([128, 512], F32,
                                  name=f"ot_{sfx}_{g}_{qc}_{hh}", tag="ot")
                    e8s = {}

                    def attnv(mip):
                        nc.tensor.matmul(
                            ot[0:65, :],
                            v8[:, g, mip, :, hh * 80:hh * 80 + 65],
                            e8s[mip][:],
                            start=(mip == 0), stop=(mip == 7),
                            perf_mode=PM.DoubleRow,
                        )

                    for mip in range(8):
                        sp = psS.tile([128, 1024], F32,
                                      name=f"sp_{sfx}_{g}_{qc}_{hh}_{mip}",
                                      tag="sc")
                        for k in range(2):
                            mi = 2 * mip + k
                            nc.tensor.matmul(
                                sp[:, k * 512:(k + 1) * 512],
                                k8[prow, g, :, mi * 128:(mi + 1) * 128],
                                q8[prow, g, :, qs],
                                start=True, stop=True,
                                perf_mode=PM.DoubleRow,
                                tile_position=(p0, 0),
                            )
                        e8 = p3.tile([128, 2, 512], F8,
                                     name=f"e8_{sfx}_{g}_{qc}_{hh}_{mip}",
                                     tag="e8")
                        e8s[mip] = e8
                        if pick_ad(1024) == "A":
                            nc.scalar.activation(
                                e8[:].rearrange("p s f -> p (s f)"),
                                sp[:], AF.Exp, scale=1.0 / TEMPER)
                        else:
                            nc.vector.tensor_scalar(
                                e8[:].bitcast(I8).rearrange("p s f -> p (s f)"),
                                sp[:], EXP_S1, EXP_S2, ALU.mult, ALU.add)
                        if mip >= 1:
                            attnv(mip - 1)
                        for _ in range(pace):
                            try:
                                next(filler)()
                            except StopIteration:
                                break
                    attnv(7)
                    # drain -> recip(denominator) -> bcast -> scale
                    stage = p2.tile([65, 512], F32,
                                    name=f"st_{sfx}_{g}_{qc}_{hh}", tag="otst")
                    ad_copy(pick_ad(512), stage[:], ot[0:65, :])
                    rc = p2.tile([1, 512], F32,
                                 name=f"rc_{sfx}_{g}_{qc}_{hh}", tag="rc")
                    nc.vector.reciprocal_approx_fast(rc[:], stage[0:1, :])
                    charge_d(512)
                    rcb = p2.tile([65, 512], F32,
                                  name=f"rcb_{sfx}_{g}_{qc}_{hh}", tag="rcb")
                    nc.gpsimd.partition_broadcast(rcb[:], rc[:])
                    h = 4 * g + hh
                    nc.gpsimd.tensor_tensor(
                        ct8[:, h // 2, h % 2, qs], stage[:], rcb[:], ALU.mult)

                # ---- emission: proj(0) upfront, then attention with PE
                # filler from the next group's projections / output proj ----
                for f in proj_items(0):
                    f()
                for g in range(G):
                    if g < G - 1:
                        fill_list = proj_items(g + 1)
                    else:
                        fill_list = []
                    filler = iter(fill_list)
                    n_slots = 64 if g < G - 1 else 32
                    pace = max(1, (len(fill_list) + n_slots - 1) // n_slots)
                    for qc in range(2):
                        if g == G - 1 and qc == 1:
                            fill_list = outproj_items(range(4))
                            filler = iter(fill_list)
                            pace = max(1, (len(fill_list) + 31) // 32)
                        for hh in range(4):
                            emit_head(g, qc, hh, filler, pace)
                    for f in filler:
                        f()

                # ---- tail: output proj qt 4-7 + layernorm chain ----
                for f in outproj_items(range(4, 8)):
                    f()
                nc.gpsimd.tensor_tensor(ssq8[:], ssq16[:, 0::2],
                                        ssq16[:, 1::2], ALU.add)
                nc.gpsimd.tensor_scalar(mu8[:], mu8[:], 1.0 / D, None, ALU.mult)
                nc.gpsimd.tensor_tensor(m28[:], mu8[:], mu8[:], ALU.mult)
                nc.vector.scalar_tensor_tensor(cs8[:], m28[:], -float(D),
                                               ssq8[:], ALU.mult, ALU.add)
                nc.scalar.activation(rs8[:], cs8[:], AF.Abs_reciprocal_sqrt,
                                     scale=1.0 / ((D - 1) * VSCALE * VSCALE))
                nc.gpsimd.tensor_tensor(rr8[:], rs8[:], rs8[:], ALU.mult)
                nc.vector.scalar_tensor_tensor(rec8[:], rr8[:], -LN_EPS,
                                               rs8[:], ALU.mult, ALU.add)
                for qt in range(8):
                    o_t = p2.tile([128, D], F32, name=f"o_{sfx}_{qt}", tag="o")
                    nc.gpsimd.tensor_scalar(o_t[:], y_ts[qt][:],
                                            mu8[:, qt:qt + 1],
                                            rec8[:, qt:qt + 1],
                                            ALU.subtract, ALU.mult)
                    nc.sync.dma_start(out_d[qt * 128:(qt + 1) * 128, :], o_t[:])

    nc.compile()
    return nc


def _get_nc():
    if "nc" not in _CACHE:
        _CACHE["nc"] = build()
    return _CACHE["nc"]


def _prep_shared(w_qs, w_ks, w_vs, proj_w):
    """fp8 weight layouts: rows d -> [p, j, s] with d = 256j + 128s + p."""
    def dsplit(a):  # [1024, N] -> [128, 4*2*N]
        n = a.shape[1]
        return np.ascontiguousarray(
            a.reshape(G, 2, 128, n).transpose(2, 0, 1, 3).reshape(128, -1)
        )

    # wq/wk cols: g*256 + (dk//32)*128 + hh*32 + dk%32  <- head 4g+hh
    wq = np.empty((D, H * DK), dtype=np.float32)
    wk = np.empty((D, H * DK), dtype=np.float32)
    for g in range(G):
        for s in range(2):
            for hh in range(4):
                c0 = g * 256 + s * 128 + hh * 32
                wq[:, c0:c0 + 32] = w_qs[4 * g + hh, :, 32 * s:32 * s + 32]
                wk[:, c0:c0 + 32] = w_ks[4 * g + hh, :, 32 * s:32 * s + 32]
    # wv cols: g*260 + hh*65 + (1+dv); col hh*65 is the ones slot
    wv = np.zeros((D, G * 4 * 65), dtype=np.float32)
    for g in range(G):
        for hh in range(4):
            c0 = g * 260 + hh * 65
            wv[:, c0 + 1:c0 + 65] = w_vs[4 * g + hh] * VSCALE
    # pw8 [65, 8, 2, 1024]: row p=0 zero (denominator slot), p=1+dv maps
    # to concat row (2j+s)*64+dv of proj_w.T
    pwT = proj_w.T.astype(np.float32)  # [c, o]
    pw8 = np.zeros((65, 8, 2, D), dtype=np.float32)
    for j in range(8):
        for s in range(2):
            h = 2 * j + s
            pw8[1:65, j, s, :] = pwT[h * 64:(h + 1) * 64, :]
    pw8 = pw8.reshape(65, -1)
    wq8 = dsplit(wq).astype(E4M3)
    wk8 = dsplit(wk).astype(E4M3)
    wv8 = dsplit(wv).astype(E4M3)
    pw8 = np.ascontiguousarray(pw8).astype(E4M3)
    return wq8, wk8, wv8, pw8


def kernel(q, w_qs, w_ks, w_vs, proj_w, proj_b, ln_a, ln_b, **kw):
    q = np.asarray(q, dtype=np.float32)
    w_qs = np.asarray(w_qs, dtype=np.float32)
    w_ks = np.asarray(w_ks, dtype=np.float32)
    w_vs = np.asarray(w_vs, dtype=np.float32)
    proj_w = np.asarray(proj_w, dtype=np.float32)
    proj_b = np.asarray(proj_b, dtype=np.float32)
    ln_a = np.asarray(ln_a, dtype=np.float32)
    ln_b = np.asarray(ln_b, dtype=np.float32)

    wq8, wk8, wv8, pw8 = _prep_shared(w_qs, w_ks, w_vs, proj_w)

    in_maps = []
    for c in range(8):
        b, half = c // 2, c % 2
        qbT = q[b].T  # [D, L]
        qcat = np.concatenate(
            [qbT[:, half * HALF:(half + 1) * HALF],
             qbT[:, (1 - half) * HALF:(2 - half) * HALF]], axis=1)
        qt8 = np.ascontiguousarray(
            qcat.reshape(G, 2, 128, L).transpose(2, 0, 1, 3).reshape(128, -1)
        ).astype(E4M3)
        qres_c = np.ascontiguousarray(
            (q[b, half * HALF:(half + 1) * HALF, :] + proj_b[None, :]) * VSCALE)
        in_maps.append({
            "qt8": qt8, "qres": qres_c,
            "wq8": wq8, "wk8": wk8, "wv8": wv8, "pw8": pw8,
        })

    nc = _get_nc()
    res = run_bass_kernel_spmd(nc, in_maps, core_ids=list(range(8))).results

    out = np.empty((B, L, D), dtype=np.float32)
    for c in range(8):
        b, half = c // 2, c % 2
        out[b, half * HALF:(half + 1) * HALF, :] = res[c]["out"]
    # ln affine on host
    out = out * (ln_a[None, None, :] / VSCALE) + ln_b[None, None, :]
    return out


# revision 12
# speedup vs baseline: 1.8655x; 1.0249x over previous
"""MultiHeadAttention TRN2 Bass kernel (8 NeuronCores), fp8 DoubleRow edition.

Sharding: core c = (batch b = c//2, query-half = c%2). Each core computes
K/V for its full batch (2048 keys) and attention + output projection + LN
for its 1024 query rows. No collectives; host gathers per-core outputs.

All heavy matmuls run in fp8e4m3 with MatmulPerfMode.DoubleRow (0.5 PE
cycles per output row, 256-wide contraction per instruction):
  - Q/K projections:  out[4head*32dk half, q|m] over D=1024 (4 DR chunks)
  - V projection:     Vaug[m, 4head*65] (col 0 of each head = ones so the
                      softmax denominator lands at partition 0 of OT)
  - scores:           S[m, q] per head, dk=64 = 2x32 DR slices at
                      tile_position (32*hh, 0)
  - attn*V:           OT[65, q] accumulated over 8 DR m-pair chunks
  - output proj:      Y[q, o] over 8 chunks of [65,2] (denominator row is
                      multiplied by zeroed pw rows)

PSUM can only be read by ACT and DVE, so exp(S/32), the K/Q/V fp8
converts, OT drains, and residual adds are greedily load-balanced
between those two engines at build time; Pool (gpsimd) handles all
SBUF-side work (denominator broadcast + CT scale, LN stats + finals).
exp on DVE uses the int8 bit trick:
  i8 = rne(S*8/(32*ln2) + 55.63), bitcast int8 -> fp8e4m3 ~= exp(S/32)
(the denominator sums the same approximated values -> consistent).

LayerNorm: device computes z = (y - mu)/(sigma_ddof1 + eps); the ln_a/ln_b
affine is applied on host. V is scaled x8 on host (pw /8) to keep CT
inside the fp8 normal range.
"""
import numpy as np
import ml_dtypes

import concourse.bass as bass
import concourse.mybir as mybir
import concourse.tile as tile
from concourse import bacc
from concourse.bass_utils import run_bass_kernel_spmd

F32 = mybir.dt.float32
F32R = mybir.dt.float32r
F8 = mybir.dt.float8e4
I8 = mybir.dt.int8
AF = mybir.ActivationFunctionType
ALU = mybir.AluOpType
PM = mybir.MatmulPerfMode
E4M3 = ml_dtypes.float8_e4m3

B, L, D = 4, 2048, 1024
H, DK = 16, 64
HALF = 1024            # query rows per core
TEMPER = 32.0          # sqrt(d_model)
G = 4                  # head groups of 4
LN_EPS = 1e-3
VSCALE = 8.0           # host scales w_vs by this, pw by 1/this
EXP_S1 = float(8.0 / (TEMPER * np.log(2.0)))
EXP_S2 = 56.0 - 0.37   # rne magic (calibrated on hw)

_CACHE = {}


def build(iters=1):
    nc = bacc.Bacc(None, target_bir_lowering=False)
    qt8_d = nc.dram_tensor("qt8", [128, G * 2 * L], F8, kind="ExternalInput")
    wq8_d = nc.dram_tensor("wq8", [128, G * 2 * 1024], F8, kind="ExternalInput")
    wk8_d = nc.dram_tensor("wk8", [128, G * 2 * 1024], F8, kind="ExternalInput")
    wv8_d = nc.dram_tensor("wv8", [128, G * 2 * 1040], F8, kind="ExternalInput")
    pw8_d = nc.dram_tensor("pw8", [65, 8 * 2 * 1024], F8, kind="ExternalInput")
    qres_d = nc.dram_tensor("qres", [HALF, D], F32, kind="ExternalInput")
    out_d = nc.dram_tensor("out", [HALF, D], F32, kind="ExternalOutput")

    # build-time greedy ACT/DVE balancing (ns estimates incl. seq overhead)
    eng_ns = {"A": 0.0, "D": 0.0}

    def pick_ad(rows):
        ca = rows * 0.853 + 124.0
        cd = rows * 1.065 + 108.0
        if eng_ns["A"] + ca <= eng_ns["D"] + cd:
            eng_ns["A"] += ca
            return "A"
        eng_ns["D"] += cd
        return "D"

    def charge_d(rows):
        eng_ns["D"] += rows * 1.065 + 108.0

    def ad_copy(e, dst, src):
        if e == "A":
            nc.scalar.activation(dst, src, AF.Copy)
        else:
            nc.vector.tensor_copy(dst, src)

    with tile.TileContext(nc) as tc:
        with (
            tc.tile_pool(name="p1", bufs=1) as p1,
            tc.tile_pool(name="p2", bufs=2) as p2,
            tc.tile_pool(name="p3", bufs=4) as p3,
            tc.tile_pool(name="psS", bufs=2, space="PSUM") as psS,
            tc.tile_pool(name="psO", bufs=2, space="PSUM") as psO,
            tc.tile_pool(name="psA", bufs=2, space="PSUM") as psA,
        ):
            # ---- weight loads ----
            wk8_t = p1.tile([128, G, 2, 1024], F8, name="wk8_t")
            nc.sync.dma_start(wk8_t[:], wk8_d[:])
            qt8_t = p1.tile([128, G, 2, L], F8, name="qt8_t")
            for j in range(G):
                nc.sync.dma_start(qt8_t[:, j, :, :],
                                  qt8_d[:, j * 2 * L:(j + 1) * 2 * L])
            wq8_t = p1.tile([128, G, 2, 1024], F8, name="wq8_t")
            nc.sync.dma_start(wq8_t[:], wq8_d[:])
            wv8_t = p1.tile([128, G, 2, 1040], F8, name="wv8_t")
            nc.sync.dma_start(wv8_t[:], wv8_d[:])
            pw8_t = p1.tile([65, 8, 2, 1024], F8, name="pw8_t")
            nc.sync.dma_start(pw8_t[:], pw8_d[:])

            for it in range(iters):
                sfx = f"i{it}"
                # per-group fp8 activation stores
                q8 = p1.tile([128, G, 2, HALF], F8, name=f"q8_{sfx}")
                k8 = p1.tile([128, G, 2, L], F8, name=f"k8_{sfx}")
                v8 = p1.tile([128, G, 8, 2, 320], F8, name=f"v8_{sfx}")
                ct8 = p1.tile([65, 8, 2, HALF], F8, name=f"ct8_{sfx}")
                # ones columns of v8 (col 0 of each head's 80-block);
                # the V convert copies only fill cols 1..64.
                for hh in range(4):
                    nc.gpsimd.memset(v8[:, :, :, :, hh * 80], 1.0)

                # layernorm stat tiles
                sums = p1.tile([128, 16], F32, name=f"sums_{sfx}")
                ssq16 = p1.tile([128, 16], F32, name=f"ssq16_{sfx}")
                mu8 = p1.tile([128, 8], F32, name=f"mu8_{sfx}")
                m28 = p1.tile([128, 8], F32, name=f"m28_{sfx}")
                ssq8 = p1.tile([128, 8], F32, name=f"ssq8_{sfx}")
                cs8 = p1.tile([128, 8], F32, name=f"cs8_{sfx}")
                rs8 = p1.tile([128, 8], F32, name=f"rs8_{sfx}")
                rr8 = p1.tile([128, 8], F32, name=f"rr8_{sfx}")
                rec8 = p1.tile([128, 8], F32, name=f"rec8_{sfx}")
                nmr8 = p1.tile([128, 8], F32, name=f"nmr8_{sfx}")
                y_ts = [p1.tile([128, D], F32, name=f"y_{sfx}_{qt}")
                        for qt in range(8)]

                def defer_weave(groups, lag=2):
                    """groups: list of (pe_closure, post_closure|None).
                    Weave so each post lands `lag` slots after its pe part."""
                    items = []
                    pend = []
                    for pe_f, post_f in groups:
                        items.append(pe_f)
                        pend.append(post_f)
                        if len(pend) > lag:
                            f = pend.pop(0)
                            if f is not None:
                                items.append(f)
                    for f in pend:
                        if f is not None:
                            items.append(f)
                    return items

                def proj_groups(g):
                    """K/Q/V projections for group g: (matmuls, convert)."""
                    groups = []

                    def kq(wt, dst, g, s, blk, nm):
                        hold = {}

                        def mms(hold=hold, g=g, s=s, blk=blk, nm=nm, wt=wt):
                            hold["t"] = psA.tile(
                                [128, 512], F32,
                                name=f"{nm}_{sfx}_{g}_{s}_{blk}", tag="acc")
                            for j in range(G):
                                nc.tensor.matmul(
                                    hold["t"][:],
                                    wt[:, j, :, g * 256 + s * 128:
                                       g * 256 + s * 128 + 128],
                                    qt8_t[:, j, :, blk * 512:(blk + 1) * 512],
                                    start=(j == 0), stop=(j == G - 1),
                                    perf_mode=PM.DoubleRow,
                                )

                        def cv(hold=hold, dst=dst):
                            ad_copy(pick_ad(512), dst, hold["t"][:])

                        return (mms, cv)

                    for s in range(2):
                        for mb in range(4):
                            groups.append(kq(
                                wk8_t, k8[:, g, s, mb * 512:(mb + 1) * 512],
                                g, s, mb, "kp"))
                    for s in range(2):
                        for qb in range(2):
                            groups.append(kq(
                                wq8_t, q8[:, g, s, qb * 512:(qb + 1) * 512],
                                g, s, qb, "qp"))
                    for mt in range(16):
                        hold = {}

                        def vmms(hold=hold, g=g, mt=mt):
                            hold["t"] = psA.tile(
                                [128, 512], F32,
                                name=f"vp_{sfx}_{g}_{mt}", tag="acc")
                            for j in range(G):
                                nc.tensor.matmul(
                                    hold["t"][:, 0:260],
                                    qt8_t[:, j, :, mt * 128:(mt + 1) * 128],
                                    wv8_t[:, j, :, g * 260:(g + 1) * 260],
                                    start=(j == 0), stop=(j == G - 1),
                                    perf_mode=PM.DoubleRow,
                                )

                        def vcv(hold=hold, g=g, mt=mt):
                            ad_copy(
                                pick_ad(256),
                                v8[:, g, mt // 2, mt % 2, :]
                                .rearrange("p (h f) -> p h f", h=4)[:, :, 1:65],
                                hold["t"][:, 0:260]
                                .rearrange("p (h f) -> p h f", h=4)[:, :, 1:65],
                            )

                        groups.append((vmms, vcv))
                    return defer_weave(groups)

                def outproj_groups(qts):
                    """Output projection + y-add + squares as (pe, post)."""
                    groups = []
                    for qt in qts:
                        qr_hold = {}

                        def mk_mms(qt, oc, hold, qh):
                            def mms():
                                if oc == 0:
                                    qh["t"] = p2.tile([128, D], F32,
                                                      name=f"qr_{sfx}_{qt}",
                                                      tag="qr")
                                    nc.sync.dma_start(
                                        qh["t"][:],
                                        qres_d[qt * 128:(qt + 1) * 128, :])
                                hold["t"] = psA.tile(
                                    [128, 512], F32,
                                    name=f"yp_{sfx}_{qt}_{oc}", tag="acc")
                                for j in range(8):
                                    nc.tensor.matmul(
                                        hold["t"][:],
                                        ct8[:, j, :, qt * 128:(qt + 1) * 128],
                                        pw8_t[:, j, :, oc * 512:(oc + 1) * 512],
                                        start=(j == 0), stop=(j == 7),
                                        perf_mode=PM.DoubleRow,
                                    )
                            return mms

                        def mk_post(qt, oc, hold, qh):
                            def post():
                                nc.vector.scalar_tensor_tensor(
                                    y_ts[qt][:, oc * 512:(oc + 1) * 512],
                                    hold["t"][:], 1.0,
                                    qh["t"][:, oc * 512:(oc + 1) * 512],
                                    ALU.mult, ALU.add,
                                    accum_out=sums[:, 2 * qt + oc:
                                                   2 * qt + oc + 1])
                                charge_d(512)
                                sqt = p2.tile([128, 512], F32,
                                              name=f"sqt_{sfx}_{qt}_{oc}",
                                              tag="sqt")
                                nc.scalar.activation(
                                    sqt[:], y_ts[qt][:, oc * 512:(oc + 1) * 512],
                                    AF.Square,
                                    accum_out=ssq16[:, 2 * qt + oc:
                                                    2 * qt + oc + 1])
                                eng_ns["A"] += 512 * 0.853 + 124.0
                            return post

                        for oc in range(2):
                            hold = {}
                            groups.append((mk_mms(qt, oc, hold, qr_hold),
                                           mk_post(qt, oc, hold, qr_hold)))
                    return defer_weave(groups)

                def ln_chain(lo, hi):
                    """sigma chain + finals for q tiles [lo, hi)."""
                    cl = slice(lo, hi)
                    nc.gpsimd.tensor_tensor(ssq8[:, cl],
                                            ssq16[:, 2 * lo:2 * hi:2],
                                            ssq16[:, 2 * lo + 1:2 * hi:2],
                                            ALU.add)
                    nc.gpsimd.tensor_tensor(mu8[:, cl],
                                            sums[:, 2 * lo:2 * hi:2],
                                            sums[:, 2 * lo + 1:2 * hi:2],
                                            ALU.add)
                    nc.gpsimd.tensor_scalar(mu8[:, cl], mu8[:, cl], 1.0 / D,
                                            None, ALU.mult)
                    nc.gpsimd.tensor_tensor(m28[:, cl], mu8[:, cl], mu8[:, cl],
                                            ALU.mult)
                    nc.vector.scalar_tensor_tensor(cs8[:, cl], m28[:, cl],
                                                   -float(D), ssq8[:, cl],
                                                   ALU.mult, ALU.add)
                    nc.scalar.activation(rs8[:, cl], cs8[:, cl],
                                         AF.Abs_reciprocal_sqrt,
                                         scale=1.0 / ((D - 1) * VSCALE * VSCALE))
                    nc.gpsimd.tensor_tensor(rr8[:, cl], rs8[:, cl], rs8[:, cl],
                                            ALU.mult)
                    nc.vector.scalar_tensor_tensor(rec8[:, cl], rr8[:, cl],
                                                   -LN_EPS, rs8[:, cl],
                                                   ALU.mult, ALU.add)
                    nc.gpsimd.tensor_tensor(nmr8[:, cl], mu8[:, cl],
                                            rec8[:, cl], ALU.mult)
                    nc.gpsimd.tensor_scalar(nmr8[:, cl], nmr8[:, cl], -1.0,
                                            None, ALU.mult)
                    for qt in range(lo, hi):
                        o_t = p2.tile([128, D], F32, name=f"o_{sfx}_{qt}",
                                      tag="o")
                        if pick_ad(1024) == "A":
                            nc.scalar.activation(
                                o_t[:], y_ts[qt][:], AF.Identity,
                                bias=nmr8[:, qt:qt + 1],
                                scale=rec8[:, qt:qt + 1])
                        else:
                            nc.vector.tensor_scalar(
                                o_t[:], y_ts[qt][:], mu8[:, qt:qt + 1],
                                rec8[:, qt:qt + 1], ALU.subtract, ALU.mult)
                        nc.sync.dma_start(out_d[qt * 128:(qt + 1) * 128, :],
                                          o_t[:])

                def emit_head(g, qc, hh, filler, pace):
                    qs = slice(qc * 512, (qc + 1) * 512)
                    p0 = 32 * hh
                    prow = slice(p0, p0 + 32)
                    ot = psO.tile([128, 512], F32,
                                  name=f"ot_{sfx}_{g}_{qc}_{hh}", tag="ot")
                    e8s = {}

                    def attnv(mip):
                        nc.tensor.matmul(
                            ot[0:65, :],
                            v8[:, g, mip, :, hh * 80:hh * 80 + 65],
                            e8s[mip][:],
                            start=(mip == 0), stop=(mip == 7),
                            perf_mode=PM.DoubleRow,
                        )

                    for mip in range(8):
                        sp = psS.tile([128, 1024], F32,
                                      name=f"sp_{sfx}_{g}_{qc}_{hh}_{mip}",
                                      tag="sc")
                        for k in range(2):
                            mi = 2 * mip + k
                            nc.tensor.matmul(
                                sp[:, k * 512:(k + 1) * 512],
                                k8[prow, g, :, mi * 128:(mi + 1) * 128],
                                q8[prow, g, :, qs],
                                start=True, stop=True,
                                perf_mode=PM.DoubleRow,
                                tile_position=(p0, 0),
                            )
                        e8 = p3.tile([128, 2, 512], F8,
                                     name=f"e8_{sfx}_{g}_{qc}_{hh}_{mip}",
                                     tag="e8")
                        e8s[mip] = e8
                        if pick_ad(1024) == "A":
                            nc.scalar.activation(
                                e8[:].rearrange("p s f -> p (s f)"),
                                sp[:], AF.Exp, scale=1.0 / TEMPER)
                        else:
                            nc.vector.tensor_scalar(
                                e8[:].bitcast(I8).rearrange("p s f -> p (s f)"),
                                sp[:], EXP_S1, EXP_S2, ALU.mult, ALU.add)
                        if mip >= 1:
                            attnv(mip - 1)
                        for _ in range(pace):
                            try:
                                next(filler)()
                            except StopIteration:
                                break
                    attnv(7)
                    # drain -> recip(denominator) -> bcast -> scale
                    stage = p2.tile([65, 512], F32,
                                    name=f"st_{sfx}_{g}_{qc}_{hh}", tag="otst")
                    ad_copy(pick_ad(512), stage[:], ot[0:65, :])
                    rc = p2.tile([1, 512], F32,
                                 name=f"rc_{sfx}_{g}_{qc}_{hh}", tag="rc")
                    nc.vector.reciprocal_approx_fast(rc[:], stage[0:1, :])
                    charge_d(512)
                    rcb = p2.tile([65, 512], F32,
                                  name=f"rcb_{sfx}_{g}_{qc}_{hh}", tag="rcb")
                    nc.gpsimd.partition_broadcast(rcb[:], rc[:])
                    h = 4 * g + hh
                    nc.gpsimd.tensor_tensor(
                        ct8[:, h // 2, h % 2, qs], stage[:], rcb[:], ALU.mult)

                # ---- emission: proj(0) upfront, then attention with PE
                # filler from the next group's projections / output proj ----
                for f in proj_groups(0):
                    f()
                for g in range(G):
                    fill_list = proj_groups(g + 1) if g < G - 1 else []
                    filler = iter(fill_list)
                    pace = 1
                    for qc in range(2):
                        if g == G - 1 and qc == 1:
                            fill_list = outproj_groups(range(4))
                            filler = iter(fill_list)
                        for hh in range(4):
                            emit_head(g, qc, hh, filler, pace)
                    for f in filler:
                        f()

                # ---- tail: finals for qt 0-3 overlap outproj qt 4-7 ----
                ln_chain(0, 4)
                for f in outproj_groups(range(4, 8)):
                    f()
                ln_chain(4, 8)

    nc.compile()
    return nc


def _get_nc():
    if "nc" not in _CACHE:
        _CACHE["nc"] = build()
    return _CACHE["nc"]


def _prep_shared(w_qs, w_ks, w_vs, proj_w):
    """fp8 weight layouts: rows d -> [p, j, s] with d = 256j + 128s + p."""
    def dsplit(a):  # [1024, N] -> [128, 4*2*N]
        n = a.shape[1]
        return np.ascontiguousarray(
            a.reshape(G, 2, 128, n).transpose(2, 0, 1, 3).reshape(128, -1)
        )

    # wq/wk cols: g*256 + (dk//32)*128 + hh*32 + dk%32  <- head 4g+hh
    wq = np.empty((D, H * DK), dtype=np.float32)
    wk = np.empty((D, H * DK), dtype=np.float32)
    for g in range(G):
        for s in range(2):
            for hh in range(4):
                c0 = g * 256 + s * 128 + hh * 32
                wq[:, c0:c0 + 32] = w_qs[4 * g + hh, :, 32 * s:32 * s + 32]
                wk[:, c0:c0 + 32] = w_ks[4 * g + hh, :, 32 * s:32 * s + 32]
    # wv cols: g*260 + hh*65 + (1+dv); col hh*65 is the ones slot
    wv = np.zeros((D, G * 4 * 65), dtype=np.float32)
    for g in range(G):
        for hh in range(4):
            c0 = g * 260 + hh * 65
            wv[:, c0 + 1:c0 + 65] = w_vs[4 * g + hh] * VSCALE
    # pw8 [65, 8, 2, 1024]: row p=0 zero (denominator slot), p=1+dv maps
    # to concat row (2j+s)*64+dv of proj_w.T
    pwT = proj_w.T.astype(np.float32)  # [c, o]
    pw8 = np.zeros((65, 8, 2, D), dtype=np.float32)
    for j in range(8):
        for s in range(2):
            h = 2 * j + s
            pw8[1:65, j, s, :] = pwT[h * 64:(h + 1) * 64, :]
    pw8 = pw8.reshape(65, -1)
    wq8 = dsplit(wq).astype(E4M3)
    wk8 = dsplit(wk).astype(E4M3)
    wv8 = dsplit(wv).astype(E4M3)
    pw8 = np.ascontiguousarray(pw8).astype(E4M3)
    return wq8, wk8, wv8, pw8


def kernel(q, w_qs, w_ks, w_vs, proj_w, proj_b, ln_a, ln_b, **kw):
    q = np.asarray(q, dtype=np.float32)
    w_qs = np.asarray(w_qs, dtype=np.float32)
    w_ks = np.asarray(w_ks, dtype=np.float32)
    w_vs = np.asarray(w_vs, dtype=np.float32)
    proj_w = np.asarray(proj_w, dtype=np.float32)
    proj_b = np.asarray(proj_b, dtype=np.float32)
    ln_a = np.asarray(ln_a, dtype=np.float32)
    ln_b = np.asarray(ln_b, dtype=np.float32)

    wq8, wk8, wv8, pw8 = _prep_shared(w_qs, w_ks, w_vs, proj_w)

    in_maps = []
    for c in range(8):
        b, half = c // 2, c % 2
        qbT = q[b].T  # [D, L]
        qcat = np.concatenate(
            [qbT[:, half * HALF:(half + 1) * HALF],
             qbT[:, (1 - half) * HALF:(2 - half) * HALF]], axis=1)
        qt8 = np.ascontiguousarray(
            qcat.reshape(G, 2, 128, L).transpose(2, 0, 1, 3).reshape(128, -1)
        ).astype(E4M3)
        qres_c = np.ascontiguousarray(
            (q[b, half * HALF:(half + 1) * HALF, :] + proj_b[None, :]) * VSCALE)
        in_maps.append({
            "qt8": qt8, "qres": qres_c,
            "wq8": wq8, "wk8": wk8, "wv8": wv8, "pw8": pw8,
        })

    nc = _get_nc()
    res = run_bass_kernel_spmd(nc, in_maps, core_ids=list(range(8))).results

    out = np.empty((B, L, D), dtype=np.float32)
    for c in range(8):
        b, half = c // 2, c % 2
        out[b, half * HALF:(half + 1) * HALF, :] = res[c]["out"]
    # ln affine on host
    out = out * (ln_a[None, None, :] / VSCALE) + ln_b[None, None, :]
    return out


# revision 14
# speedup vs baseline: 1.9050x; 1.0212x over previous
"""MultiHeadAttention TRN2 Bass kernel (8 NeuronCores), fp8 DoubleRow edition.

Sharding: core c = (batch b = c//2, query-half = c%2). Each core computes
K/V for its full batch (2048 keys) and attention + output projection + LN
for its 1024 query rows. No collectives; host gathers per-core outputs.

All heavy matmuls run in fp8e4m3 with MatmulPerfMode.DoubleRow (0.5 PE
cycles per output row, 256-wide contraction per instruction):
  - Q/K projections:  out[4head*32dk half, q|m] over D=1024 (4 DR chunks)
  - V projection:     Vaug[m, 4head*65] (col 0 of each head = ones so the
                      softmax denominator lands at partition 0 of OT)
  - scores:           S[m, q] per head, dk=64 = 2x32 DR slices at
                      tile_position (32*hh, 0)
  - attn*V:           OT[65, q] accumulated over 8 DR m-pair chunks
  - output proj:      Y[q, o] over 8 chunks of [65,2] (denominator row is
                      multiplied by zeroed pw rows)

PSUM can only be read by ACT and DVE, so exp(S/32), the K/Q/V fp8
converts, OT drains, and residual adds are greedily load-balanced
between those two engines at build time; Pool (gpsimd) handles all
SBUF-side work (denominator broadcast + CT scale, LN stats + finals).
exp on DVE uses the int8 bit trick:
  i8 = rne(S*8/(32*ln2) + 55.63), bitcast int8 -> fp8e4m3 ~= exp(S/32)
(the denominator sums the same approximated values -> consistent).

LayerNorm: device computes z = (y - mu)/(sigma_ddof1 + eps); the ln_a/ln_b
affine is applied on host. V is scaled x8 on host (pw /8) to keep CT
inside the fp8 normal range.
"""
import numpy as np
import ml_dtypes

import concourse.bass as bass
import concourse.mybir as mybir
import concourse.tile as tile
from concourse import bacc
from concourse.bass_utils import run_bass_kernel_spmd

F32 = mybir.dt.float32
F32R = mybir.dt.float32r
F8 = mybir.dt.float8e4
I8 = mybir.dt.int8
I32 = mybir.dt.int32
AF = mybir.ActivationFunctionType
ALU = mybir.AluOpType
PM = mybir.MatmulPerfMode
E4M3 = ml_dtypes.float8_e4m3

B, L, D = 4, 2048, 1024
H, DK = 16, 64
HALF = 1024            # query rows per core
TEMPER = 32.0          # sqrt(d_model)
G = 4                  # head groups of 4
LN_EPS = 1e-3
VSCALE = 8.0           # host scales w_vs by this, pw by 1/this
EXP_S1 = float(8.0 / (TEMPER * np.log(2.0)))
EXP_S2 = 56.0 - 0.37   # rne magic (calibrated on hw)
MAGIC_RCP = 0x7EF30000   # reciprocal seed; 1 Newton -> 0.26% max err
MAGIC_RSQ = 0x5F3759DF   # rsqrt seed; 2 Newtons -> 5e-6

_CACHE = {}


def build(iters=1):
    nc = bacc.Bacc(None, target_bir_lowering=False)
    qt8_d = nc.dram_tensor("qt8", [128, G * 2 * L], F8, kind="ExternalInput")
    wq8_d = nc.dram_tensor("wq8", [128, G * 2 * 1024], F8, kind="ExternalInput")
    wk8_d = nc.dram_tensor("wk8", [128, G * 2 * 1024], F8, kind="ExternalInput")
    wv8_d = nc.dram_tensor("wv8", [128, G * 2 * 1040], F8, kind="ExternalInput")
    pw8_d = nc.dram_tensor("pw8", [65, 8 * 2 * 1024], F8, kind="ExternalInput")
    qres_d = nc.dram_tensor("qres", [HALF, D], F32, kind="ExternalInput")
    out_d = nc.dram_tensor("out", [HALF, D], F32, kind="ExternalOutput")

    # build-time greedy ACT/DVE balancing (ns estimates incl. seq overhead)
    eng_ns = {"A": 0.0, "D": 0.0}

    def pick_ad(rows):
        ca = rows * 0.853 + 124.0
        cd = rows * 1.065 + 108.0
        if eng_ns["A"] + ca <= eng_ns["D"] + cd:
            eng_ns["A"] += ca
            return "A"
        eng_ns["D"] += cd
        return "D"

    def charge_d(rows):
        eng_ns["D"] += rows * 1.065 + 108.0

    def ad_copy(e, dst, src):
        if e == "A":
            nc.scalar.activation(dst, src, AF.Copy)
        else:
            nc.vector.tensor_copy(dst, src)

    with tile.TileContext(nc) as tc:
        with (
            tc.tile_pool(name="p1", bufs=1) as p1,
            tc.tile_pool(name="p2", bufs=2) as p2,
            tc.tile_pool(name="p3", bufs=4) as p3,
            tc.tile_pool(name="psS", bufs=2, space="PSUM") as psS,
            tc.tile_pool(name="psO", bufs=2, space="PSUM") as psO,
            tc.tile_pool(name="psA", bufs=2, space="PSUM") as psA,
        ):
            # ---- weight loads ----
            wk8_t = p1.tile([128, G, 2, 1024], F8, name="wk8_t")
            nc.sync.dma_start(wk8_t[:], wk8_d[:])
            qt8_t = p1.tile([128, G, 2, L], F8, name="qt8_t")
            for j in range(G):
                nc.scalar.dma_start(qt8_t[:, j, :, :],
                                    qt8_d[:, j * 2 * L:(j + 1) * 2 * L])
            wq8_t = p1.tile([128, G, 2, 1024], F8, name="wq8_t")
            nc.sync.dma_start(wq8_t[:], wq8_d[:])
            wv8_t = p1.tile([128, G, 2, 1040], F8, name="wv8_t")
            nc.sync.dma_start(wv8_t[:], wv8_d[:])
            pw8_t = p1.tile([65, 8, 2, 1024], F8, name="pw8_t")
            nc.sync.dma_start(pw8_t[:], pw8_d[:])

            for it in range(iters):
                sfx = f"i{it}"
                # per-group fp8 activation stores
                q8 = p1.tile([128, G, 2, HALF], F8, name=f"q8_{sfx}")
                k8 = p1.tile([128, G, 2, L], F8, name=f"k8_{sfx}")
                v8 = p1.tile([128, G, 8, 2, 320], F8, name=f"v8_{sfx}")
                ct8 = p1.tile([65, 8, 2, HALF], F8, name=f"ct8_{sfx}")
                # ones columns of v8 (col 0 of each head's 80-block);
                # the V convert copies only fill cols 1..64.
                for hh in range(4):
                    nc.gpsimd.memset(v8[:, :, :, :, hh * 80], 1.0)

                # layernorm stat tiles
                sums = p1.tile([128, 16], F32, name=f"sums_{sfx}")
                ssq16 = p1.tile([128, 16], F32, name=f"ssq16_{sfx}")
                mu8 = p1.tile([128, 8], F32, name=f"mu8_{sfx}")
                m28 = p1.tile([128, 8], F32, name=f"m28_{sfx}")
                ssq8 = p1.tile([128, 8], F32, name=f"ssq8_{sfx}")
                cs8 = p1.tile([128, 8], F32, name=f"cs8_{sfx}")
                var8 = p1.tile([128, 8], F32, name=f"var8_{sfx}")
                si8 = p1.tile([128, 8], I32, name=f"si8_{sfx}")
                a8 = p1.tile([128, 8], F32, name=f"a8_{sfx}")
                b8 = p1.tile([128, 8], F32, name=f"b8_{sfx}")
                rs8 = p1.tile([128, 8], F32, name=f"rs8_{sfx}")
                rr8 = p1.tile([128, 8], F32, name=f"rr8_{sfx}")
                rec8 = p1.tile([128, 8], F32, name=f"rec8_{sfx}")
                nmr8 = p1.tile([128, 8], F32, name=f"nmr8_{sfx}")
                y_ts = [p1.tile([128, D], F32, name=f"y_{sfx}_{qt}")
                        for qt in range(8)]

                def defer_weave(groups, lag=2):
                    """groups: list of (pe_closure, post_closure|None).
                    Weave so each post lands `lag` slots after its pe part."""
                    items = []
                    pend = []
                    for pe_f, post_f in groups:
                        items.append(pe_f)
                        pend.append(post_f)
                        if len(pend) > lag:
                            f = pend.pop(0)
                            if f is not None:
                                items.append(f)
                    for f in pend:
                        if f is not None:
                            items.append(f)
                    return items

                def proj_groups(g):
                    """K/Q/V projections for group g: (matmuls, convert)."""
                    groups = []

                    def kq(wt, dst, g, s, blk, nm):
                        hold = {}

                        def mms(hold=hold, g=g, s=s, blk=blk, nm=nm, wt=wt):
                            hold["t"] = psA.tile(
                                [128, 512], F32,
                                name=f"{nm}_{sfx}_{g}_{s}_{blk}", tag="acc")
                            for j in range(G):
                                nc.tensor.matmul(
                                    hold["t"][:],
                                    wt[:, j, :, g * 256 + s * 128:
                                       g * 256 + s * 128 + 128],
                                    qt8_t[:, j, :, blk * 512:(blk + 1) * 512],
                                    start=(j == 0), stop=(j == G - 1),
                                    perf_mode=PM.DoubleRow,
                                )

                        def cv(hold=hold, dst=dst):
                            ad_copy(pick_ad(512), dst, hold["t"][:])

                        return (mms, cv)

                    for s in range(2):
                        for mb in range(4):
                            groups.append(kq(
                                wk8_t, k8[:, g, s, mb * 512:(mb + 1) * 512],
                                g, s, mb, "kp"))
                    for s in range(2):
                        for qb in range(2):
                            groups.append(kq(
                                wq8_t, q8[:, g, s, qb * 512:(qb + 1) * 512],
                                g, s, qb, "qp"))
                    for mt in range(16):
                        hold = {}

                        def vmms(hold=hold, g=g, mt=mt):
                            hold["t"] = psA.tile(
                                [128, 512], F32,
                                name=f"vp_{sfx}_{g}_{mt}", tag="acc")
                            for j in range(G):
                                nc.tensor.matmul(
                                    hold["t"][:, 0:260],
                                    qt8_t[:, j, :, mt * 128:(mt + 1) * 128],
                                    wv8_t[:, j, :, g * 260:(g + 1) * 260],
                                    start=(j == 0), stop=(j == G - 1),
                                    perf_mode=PM.DoubleRow,
                                )

                        def vcv(hold=hold, g=g, mt=mt):
                            ad_copy(
                                pick_ad(256),
                                v8[:, g, mt // 2, mt % 2, :]
                                .rearrange("p (h f) -> p h f", h=4)[:, :, 1:65],
                                hold["t"][:, 0:260]
                                .rearrange("p (h f) -> p h f", h=4)[:, :, 1:65],
                            )

                        groups.append((vmms, vcv))
                    return defer_weave(groups)

                def outproj_groups(qts):
                    """Output projection + y-add + squares as (pe, post)."""
                    groups = []
                    for qt in qts:
                        qr_hold = {}

                        def mk_mms(qt, oc, hold, qh):
                            def mms():
                                if oc == 0:
                                    qh["t"] = p2.tile([128, D], F32,
                                                      name=f"qr_{sfx}_{qt}",
                                                      tag="qr")
                                    nc.sync.dma_start(
                                        qh["t"][:],
                                        qres_d[qt * 128:(qt + 1) * 128, :])
                                hold["t"] = psA.tile(
                                    [128, 512], F32,
                                    name=f"yp_{sfx}_{qt}_{oc}", tag="acc")
                                for j in range(8):
                                    nc.tensor.matmul(
                                        hold["t"][:],
                                        ct8[:, j, :, qt * 128:(qt + 1) * 128],
                                        pw8_t[:, j, :, oc * 512:(oc + 1) * 512],
                                        start=(j == 0), stop=(j == 7),
                                        perf_mode=PM.DoubleRow,
                                    )
                            return mms

                        def mk_post(qt, oc, hold, qh):
                            def post():
                                nc.vector.scalar_tensor_tensor(
                                    y_ts[qt][:, oc * 512:(oc + 1) * 512],
                                    hold["t"][:], 1.0,
                                    qh["t"][:, oc * 512:(oc + 1) * 512],
                                    ALU.mult, ALU.add,
                                    accum_out=sums[:, 2 * qt + oc:
                                                   2 * qt + oc + 1])
                                charge_d(512)
                                sqt = p2.tile([128, 512], F32,
                                              name=f"sqt_{sfx}_{qt}_{oc}",
                                              tag="sqt")
                                yv = y_ts[qt][:, oc * 512:(oc + 1) * 512]
                                if pick_ad(512) == "A":
                                    nc.scalar.activation(
                                        sqt[:], yv, AF.Square,
                                        accum_out=ssq16[:, 2 * qt + oc:
                                                        2 * qt + oc + 1])
                                else:
                                    nc.vector.scalar_tensor_tensor(
                                        sqt[:], yv, 1.0, yv,
                                        ALU.mult, ALU.mult,
                                        accum_out=ssq16[:, 2 * qt + oc:
                                                        2 * qt + oc + 1])
                            return post

                        for oc in range(2):
                            hold = {}
                            groups.append((mk_mms(qt, oc, hold, qr_hold),
                                           mk_post(qt, oc, hold, qr_hold)))
                    return defer_weave(groups)

                def ln_chain(lo, hi):
                    """sigma chain + finals for q tiles [lo, hi)."""
                    cl = slice(lo, hi)
                    nc.gpsimd.tensor_tensor(ssq8[:, cl],
                                            ssq16[:, 2 * lo:2 * hi:2],
                                            ssq16[:, 2 * lo + 1:2 * hi:2],
                                            ALU.add)
                    nc.gpsimd.tensor_tensor(mu8[:, cl],
                                            sums[:, 2 * lo:2 * hi:2],
                                            sums[:, 2 * lo + 1:2 * hi:2],
                                            ALU.add)
                    nc.gpsimd.tensor_scalar(mu8[:, cl], mu8[:, cl], 1.0 / D,
                                            None, ALU.mult)
                    nc.gpsimd.tensor_tensor(m28[:, cl], mu8[:, cl], mu8[:, cl],
                                            ALU.mult)
                    nc.vector.scalar_tensor_tensor(cs8[:, cl], m28[:, cl],
                                                   -float(D), ssq8[:, cl],
                                                   ALU.mult, ALU.add)
                    nc.gpsimd.tensor_scalar(
                        var8[:, cl], cs8[:, cl],
                        1.0 / ((D - 1) * VSCALE * VSCALE), None, ALU.mult)
                    nc.vector.tensor_scalar(si8[:, cl],
                                            var8[:, cl].bitcast(I32), 1, None,
                                            ALU.arith_shift_right)
                    nc.vector.tensor_scalar(si8[:, cl], si8[:, cl], -1,
                                            MAGIC_RSQ, ALU.mult, ALU.add)
                    r_ap = si8[:, cl].bitcast(F32)
                    nc.gpsimd.tensor_tensor(a8[:, cl], r_ap, r_ap, ALU.mult)
                    nc.gpsimd.tensor_tensor(a8[:, cl], var8[:, cl], a8[:, cl],
                                            ALU.mult)
                    nc.gpsimd.tensor_scalar(a8[:, cl], a8[:, cl], -0.5, 1.5,
                                            ALU.mult, ALU.add)
                    nc.gpsimd.tensor_tensor(b8[:, cl], r_ap, a8[:, cl],
                                            ALU.mult)
                    nc.gpsimd.tensor_tensor(a8[:, cl], b8[:, cl], b8[:, cl],
                                            ALU.mult)
                    nc.gpsimd.tensor_tensor(a8[:, cl], var8[:, cl], a8[:, cl],
                                            ALU.mult)
                    nc.gpsimd.tensor_scalar(a8[:, cl], a8[:, cl], -0.5, 1.5,
                                            ALU.mult, ALU.add)
                    nc.gpsimd.tensor_tensor(rs8[:, cl], b8[:, cl], a8[:, cl],
                                            ALU.mult)
                    nc.gpsimd.tensor_tensor(rr8[:, cl], rs8[:, cl], rs8[:, cl],
                                            ALU.mult)
                    nc.vector.scalar_tensor_tensor(rec8[:, cl], rr8[:, cl],
                                                   -LN_EPS, rs8[:, cl],
                                                   ALU.mult, ALU.add)
                    nc.gpsimd.tensor_tensor(nmr8[:, cl], mu8[:, cl],
                                            rec8[:, cl], ALU.mult)
                    nc.gpsimd.tensor_scalar(nmr8[:, cl], nmr8[:, cl], -1.0,
                                            None, ALU.mult)
                    for qt in range(lo, hi):
                        o_t = p2.tile([128, D], F32, name=f"o_{sfx}_{qt}",
                                      tag="o")
                        if pick_ad(1024) == "A":
                            nc.scalar.activation(
                                o_t[:], y_ts[qt][:], AF.Identity,
                                bias=nmr8[:, qt:qt + 1],
                                scale=rec8[:, qt:qt + 1])
                        else:
                            nc.vector.tensor_scalar(
                                o_t[:], y_ts[qt][:], mu8[:, qt:qt + 1],
                                rec8[:, qt:qt + 1], ALU.subtract, ALU.mult)
                        nc.sync.dma_start(out_d[qt * 128:(qt + 1) * 128, :],
                                          o_t[:])

                def emit_head(g, qc, hh, filler, pace):
                    qs = slice(qc * 512, (qc + 1) * 512)
                    p0 = 32 * hh
                    prow = slice(p0, p0 + 32)
                    ot = psO.tile([128, 512], F32,
                                  name=f"ot_{sfx}_{g}_{qc}_{hh}", tag="ot")
                    e8s = {}

                    def attnv(mip):
                        nc.tensor.matmul(
                            ot[0:65, :],
                            v8[:, g, mip, :, hh * 80:hh * 80 + 65],
                            e8s[mip][:],
                            start=(mip == 0), stop=(mip == 7),
                            perf_mode=PM.DoubleRow,
                        )

                    for mip in range(8):
                        sp = psS.tile([128, 1024], F32,
                                      name=f"sp_{sfx}_{g}_{qc}_{hh}_{mip}",
                                      tag="sc")
                        for k in range(2):
                            mi = 2 * mip + k
                            nc.tensor.matmul(
                                sp[:, k * 512:(k + 1) * 512],
                                k8[prow, g, :, mi * 128:(mi + 1) * 128],
                                q8[prow, g, :, qs],
                                start=True, stop=True,
                                perf_mode=PM.DoubleRow,
                                tile_position=(p0, 0),
                            )
                        e8 = p3.tile([128, 2, 512], F8,
                                     name=f"e8_{sfx}_{g}_{qc}_{hh}_{mip}",
                                     tag="e8")
                        e8s[mip] = e8
                        if pick_ad(1024) == "A":
                            nc.scalar.activation(
                                e8[:].rearrange("p s f -> p (s f)"),
                                sp[:], AF.Exp, scale=1.0 / TEMPER)
                        else:
                            nc.vector.tensor_scalar(
                                e8[:].bitcast(I8).rearrange("p s f -> p (s f)"),
                                sp[:], EXP_S1, EXP_S2, ALU.mult, ALU.add)
                        if mip >= 1:
                            attnv(mip - 1)
                        for _ in range(pace):
                            try:
                                next(filler)()
                            except StopIteration:
                                break
                    attnv(7)
                    # drain -> recip(denominator) -> bcast -> scale
                    stage = p2.tile([65, 512], F32,
                                    name=f"st_{sfx}_{g}_{qc}_{hh}", tag="otst")
                    ad_copy(pick_ad(512), stage[:], ot[0:65, :])
                    rci = p2.tile([1, 512], I32,
                                  name=f"rci_{sfx}_{g}_{qc}_{hh}", tag="rci")
                    nc.gpsimd.tensor_scalar(rci[:], stage[0:1, :].bitcast(I32),
                                            -1, MAGIC_RCP, ALU.mult, ALU.add)
                    tt = p2.tile([1, 512], F32,
                                 name=f"tt_{sfx}_{g}_{qc}_{hh}", tag="tt")
                    nc.gpsimd.tensor_tensor(tt[:], stage[0:1, :],
                                            rci[:].bitcast(F32), ALU.mult)
                    nc.gpsimd.tensor_scalar(tt[:], tt[:], -1.0, 2.0,
                                            ALU.mult, ALU.add)
                    rc = p2.tile([1, 512], F32,
                                 name=f"rc_{sfx}_{g}_{qc}_{hh}", tag="rc")
                    nc.gpsimd.tensor_tensor(rc[:], rci[:].bitcast(F32), tt[:],
                                            ALU.mult)
                    rcb = p2.tile([65, 512], F32,
                                  name=f"rcb_{sfx}_{g}_{qc}_{hh}", tag="rcb")
                    nc.gpsimd.partition_broadcast(rcb[:], rc[:])
                    h = 4 * g + hh
                    nc.gpsimd.tensor_tensor(
                        ct8[:, h // 2, h % 2, qs], stage[:], rcb[:], ALU.mult)

                # ---- emission: proj(0) upfront, then attention with PE
                # filler from the next group's projections / output proj ----
                for f in proj_groups(0):
                    f()
                for g in range(G):
                    fill_list = proj_groups(g + 1) if g < G - 1 else []
                    filler = iter(fill_list)
                    pace = 1
                    for qc in range(2):
                        if g == G - 1 and qc == 1:
                            fill_list = outproj_groups(range(4))
                            filler = iter(fill_list)
                        for hh in range(4):
                            emit_head(g, qc, hh, filler, pace)
                    for f in filler:
                        f()

                # ---- tail: finals for qt 0-3 overlap outproj qt 4-7 ----
                ln_chain(0, 4)
                for f in outproj_groups(range(4, 8)):
                    f()
                ln_chain(4, 8)

    nc.compile()
    return nc


def _get_nc():
    if "nc" not in _CACHE:
        _CACHE["nc"] = build()
    return _CACHE["nc"]


def _prep_shared(w_qs, w_ks, w_vs, proj_w):
    """fp8 weight layouts: rows d -> [p, j, s] with d = 256j + 128s + p."""
    def dsplit(a):  # [1024, N] -> [128, 4*2*N]
        n = a.shape[1]
        return np.ascontiguousarray(
            a.reshape(G, 2, 128, n).transpose(2, 0, 1, 3).reshape(128, -1)
        )

    # wq/wk cols: g*256 + (dk//32)*128 + hh*32 + dk%32  <- head 4g+hh
    wq = np.empty((D, H * DK), dtype=np.float32)
    wk = np.empty((D, H * DK), dtype=np.float32)
    for g in range(G):
        for s in range(2):
            for hh in range(4):
                c0 = g * 256 + s * 128 + hh * 32
                wq[:, c0:c0 + 32] = w_qs[4 * g + hh, :, 32 * s:32 * s + 32]
                wk[:, c0:c0 + 32] = w_ks[4 * g + hh, :, 32 * s:32 * s + 32]
    # wv cols: g*260 + hh*65 + (1+dv); col hh*65 is the ones slot
    wv = np.zeros((D, G * 4 * 65), dtype=np.float32)
    for g in range(G):
        for hh in range(4):
            c0 = g * 260 + hh * 65
            wv[:, c0 + 1:c0 + 65] = w_vs[4 * g + hh] * VSCALE
    # pw8 [65, 8, 2, 1024]: row p=0 zero (denominator slot), p=1+dv maps
    # to concat row (2j+s)*64+dv of proj_w.T
    pwT = proj_w.T.astype(np.float32)  # [c, o]
    pw8 = np.zeros((65, 8, 2, D), dtype=np.float32)
    for j in range(8):
        for s in range(2):
            h = 2 * j + s
            pw8[1:65, j, s, :] = pwT[h * 64:(h + 1) * 64, :]
    pw8 = pw8.reshape(65, -1)
    wq8 = dsplit(wq).astype(E4M3)
    wk8 = dsplit(wk).astype(E4M3)
    wv8 = dsplit(wv).astype(E4M3)
    pw8 = np.ascontiguousarray(pw8).astype(E4M3)
    return wq8, wk8, wv8, pw8


def kernel(q, w_qs, w_ks, w_vs, proj_w, proj_b, ln_a, ln_b, **kw):
    q = np.asarray(q, dtype=np.float32)
    w_qs = np.asarray(w_qs, dtype=np.float32)
    w_ks = np.asarray(w_ks, dtype=np.float32)
    w_vs = np.asarray(w_vs, dtype=np.float32)
    proj_w = np.asarray(proj_w, dtype=np.float32)
    proj_b = np.asarray(proj_b, dtype=np.float32)
    ln_a = np.asarray(ln_a, dtype=np.float32)
    ln_b = np.asarray(ln_b, dtype=np.float32)

    wq8, wk8, wv8, pw8 = _prep_shared(w_qs, w_ks, w_vs, proj_w)

    in_maps = []
    for c in range(8):
        b, half = c // 2, c % 2
        qbT = q[b].T  # [D, L]
        qcat = np.concatenate(
            [qbT[:, half * HALF:(half + 1) * HALF],
             qbT[:, (1 - half) * HALF:(2 - half) * HALF]], axis=1)
        qt8 = np.ascontiguousarray(
            qcat.reshape(G, 2, 128, L).transpose(2, 0, 1, 3).reshape(128, -1)
        ).astype(E4M3)
        qres_c = np.ascontiguousarray(
            (q[b, half * HALF:(half + 1) * HALF, :] + proj_b[None, :]) * VSCALE)
        in_maps.append({
            "qt8": qt8, "qres": qres_c,
            "wq8": wq8, "wk8": wk8, "wv8": wv8, "pw8": pw8,
        })

    nc = _get_nc()
    res = run_bass_kernel_spmd(nc, in_maps, core_ids=list(range(8))).results

    out = np.empty((B, L, D), dtype=np.float32)
    for c in range(8):
        b, half = c // 2, c % 2
        out[b, half * HALF:(half + 1) * HALF, :] = res[c]["out"]
    # ln affine on host
    out = out * (ln_a[None, None, :] / VSCALE) + ln_b[None, None, :]
    return out
